# revision 12
# baseline (speedup 1.0000x reference)
"""Trainium2 Bass kernel v3 for nn_DCGAN_C (DCGAN conv backbone + relation net).

Key changes vs v2 (327.9us -> target ~150us):
- RN layers use dual-plane fp8 DoubleRow matmuls: each fp16 weight W is split
  host-side into W_hi = fp8(W*S) and W_lo = fp8(W*S - W_hi) and packed as the
  two DoubleRow K-planes; the moving operand is broadcast (stride-0 plane), so
  out = (W_hi + W_lo) @ x at 0.5 cyc/row with ~0.1% effective weight error.
  The PSUM carries factor S; evictions apply 1/S (ACT scale= / DVE mult-max).
  The L1 A+C expansion uses the same trick with an on-chip hi/lo split of AC.
- conv1's second (j=1, x-shifted) H1 copy is made by one SBUF->SBUF DMA per
  block instead of per-sample engine evictions.
- BN statistics via DVE bn_stats/bn_aggr (single pass, no Square pass).
- conv3 output stays resident in PSUM per mc-half; only channel stats (DVE)
  and the core's own 4 samples (ENC) are ever evicted.
- No collectives: every core runs the full-batch conv backbone (exact BN
  batch stats), RN only on its own 4 samples. Pure SPMD.
- `loop_n` wraps the body in tc.For_i for steady-state benchmarking.
"""
import os
import numpy as np
from contextlib import ExitStack

NCORES = 8
B = 32           # full batch (replicated conv backbone)
BPC = 4          # samples per core for the relation network
NUM = 64
AUX = 10
EPS = 1e-5
S_G = 8.0        # RN weight pre-scale for the fp8 hi/lo split
INV_SG = 1.0 / S_G
S_IMG = 4.0      # image pre-scale into fp8
S_C1 = 10.0      # conv weight pre-scales (sigma 0.05 -> 0.5)
S_C2 = 10.0
S_C3 = 10.0
S_H1 = 4.0       # staged activation scales (fp8 sweet spot)
S_H2 = 4.0
SC1EV = S_H1 / (S_IMG * S_C1)       # conv1 eviction scale
EPS2 = EPS * (S_H1 * S_C2) ** 2     # eps in scaled-psum units
EPS3 = EPS * (S_H2 * S_C3) ** 2

_CACHE = {}

# fp16 const-pack layout: (name, rows, width)
_P16 = [
    ("w1t", 128, 2 * 256),       # [p][kc*256 + c]
    ("w1b", 128, 2 * 256),
    ("exA", 2, 256),
    ("exC", 11, 256),
    ("aro", 2, 64),              # row0 arange, row1 ones
    ("fw1", 128, 4 * 128),
    ("fw2", 128, 4 * 128),
    ("fw3", 128, 2),
]
# fp8e4 pack: dual hi/lo (pl-major within each 2*M slice)
# c1w8: per m in 2: [j*48+dy*12+ic][m*128 + pl*64 + oc]        (M=64)
# c2w8: per (dy,m): [j*64+ic][(dy*2+m)*256 + pl*128 + oc]      (M=128)
# c3w8: per (dy,dx,mc): [ic][((dy*4+dx)*2+mc)*256 + pl*128+oc] (M=128)
# gw*:  per (kc,mc):  [(kc*2+mc)*256 + pl*128 + m]
_P8 = [("c1w8", 256), ("c2w8", 2048), ("c3w8", 8192),
       ("gw2", 1024), ("gw3", 1024), ("gw4", 1024), ("gsel8", NUM * NUM)]
_P8_OFF = {}
_off8 = 0
for _nm, _w in _P8:
    _P8_OFF[_nm] = _off8
    _off8 += _w
P8_W = _off8
_P16_OFF = {}
_off = 0
for _nm, _r, _w in _P16:
    _P16_OFF[_nm] = _off
    _off += _w
P16_W = _off
# fp32 pack [32, 204]: consts rows 0-19 cols 0-128; ident [0:32,128:160];
# fb3 [0,160]; aux [0:BPC, 161:171]
P32_W = 204


def _dual8(W, scale):
    """Split W*scale into fp8 hi + lo planes (returned as fp8 arrays)."""
    import ml_dtypes
    f8 = ml_dtypes.float8_e4m3
    hi = (W * scale).astype(f8)
    lo = (W * scale - hi.astype(np.float32)).astype(f8)
    return hi, lo


def _host_pack(inputs):
    f16 = np.float16
    w1 = inputs['conv1_w'].astype(np.float32)
    w2 = inputs['conv2_w'].astype(np.float32)
    w3 = inputs['conv3_w'].astype(np.float32)
    gW1 = inputs['gW1'].astype(np.float32)
    gb1 = inputs['gb1'].astype(np.float32)
    # the dual-fp8 DVE eviction path drops layer biases (they are zero for
    # this model); assert that stays true.
    for nm in ('gb2', 'gb3', 'gb4'):
        assert np.abs(np.asarray(inputs[nm])).max() == 0.0, nm

    pk = np.zeros((128, P16_W), f16)

    def put(nm, arr):
        o = _P16_OFF[nm]
        r, w = arr.shape
        pk[0:r, o:o + w] = arr.astype(f16)

    # conv1 K=96: rows p = j*48 + dy*12 + ic ; cols m*128 + half*64 + oc
    c1 = np.zeros((96, 256), np.float32)
    for j in range(2):
        for dy in range(4):
            for m in range(2):
                blk = w1[:, :, dy, 2 * m + j].T
                r = j * 48 + dy * 12
                c1[r:r + 12, m * 128:m * 128 + 64] = blk
                c1[r:r + 12, m * 128 + 64:m * 128 + 128] = blk
    put("c1w", c1)

    c2 = np.zeros((128, 8 * 128), np.float32)
    for dy in range(4):
        for m in range(2):
            o = (dy * 2 + m) * 128
            for j in range(2):
                c2[j * 64:(j + 1) * 64, o:o + 128] = w2[:, :, dy, 2 * m + j].T
    put("c2w", c2)

    c3 = np.zeros((128, 32 * 128), np.float32)
    for dy in range(4):
        for dx in range(4):
            for mc in range(2):
                o = ((dy * 4 + dx) * 2 + mc) * 128
                c3[:, o:o + 128] = w3[mc * 128:(mc + 1) * 128, :, dy, dx].T
    put("c3w", c3)

    w1t = np.zeros((128, 512), np.float32)
    w1b = np.zeros((128, 512), np.float32)
    for kc in range(2):
        w1t[:, kc * 256:(kc + 1) * 256] = gW1[kc * 128:(kc + 1) * 128]
        w1b[:, kc * 256:(kc + 1) * 256] = gW1[257 + kc * 128:257 + (kc + 1) * 128]
    put("w1t", w1t)
    put("w1b", w1b)
    put("exA", np.stack([gW1[256], gb1], 0))
    put("exC", np.concatenate([gW1[513:523], gW1[523:524]], 0))
    put("aro", np.stack([np.arange(64, dtype=np.float32),
                         np.ones(64, np.float32)], 0))

    def sq(nm, W):
        o = np.zeros((128, 512), np.float32)
        for kc in range(2):
            for mc in range(2):
                o[:, (kc * 2 + mc) * 128:(kc * 2 + mc + 1) * 128] = \
                    W[kc * 128:(kc + 1) * 128, mc * 128:(mc + 1) * 128]
        put(nm, o)

    sq("gw2", inputs['gW2']); sq("gw3", inputs['gW3']); sq("gw4", inputs['gW4'])
    sq("fw1", inputs['fW1']); sq("fw2", inputs['fW2'])
    f3 = np.zeros((128, 2), np.float32)
    for kc in range(2):
        f3[:, kc] = inputs['fW3'][kc * 128:(kc + 1) * 128, 0]
    put("fw3", f3)

    G = np.zeros((128, NUM * NUM), np.float32)
    ii, jj = np.meshgrid(np.arange(NUM), np.arange(NUM), indexing='ij')
    p = (ii * NUM + jj).ravel()
    G[jj.ravel(), p] = 1.0
    G[64 + ii.ravel(), p] = 1.0
    put("gsel", G)

    import ml_dtypes
    pk8 = np.zeros((128, P8_W), ml_dtypes.float8_e4m3)
    for nm, W in (("gw2", inputs['gW2']), ("gw3", inputs['gW3']),
                  ("gw4", inputs['gW4'])):
        o = _P8_OFF[nm]
        W = np.asarray(W, np.float32)
        for kc in range(2):
            for mc in range(2):
                hi, lo = _dual8(W[kc * 128:(kc + 1) * 128,
                                  mc * 128:(mc + 1) * 128], S_G)
                c0 = o + (kc * 2 + mc) * 256
                pk8[:, c0:c0 + 128] = hi
                pk8[:, c0 + 128:c0 + 256] = lo
    pk8[:, _P8_OFF["gsel8"]:_P8_OFF["gsel8"] + NUM * NUM] = \
        G.astype(ml_dtypes.float8_e4m3)

    pk32 = np.zeros((32, P32_W), np.float32)
    pk32[0, 0:128] = inputs['bn2_g']; pk32[1, 0:128] = inputs['bn2_b']
    pk32[2, 0:128] = inputs['bn3_g'][:128]; pk32[3, 0:128] = inputs['bn3_g'][128:]
    pk32[4, 0:128] = inputs['bn3_b'][:128]; pk32[5, 0:128] = inputs['bn3_b'][128:]
    pk32[6, 0:128] = inputs['gb2'][:128]; pk32[7, 0:128] = inputs['gb2'][128:]
    pk32[8, 0:128] = inputs['gb3'][:128]; pk32[9, 0:128] = inputs['gb3'][128:]
    pk32[10, 0:128] = inputs['gb4'][:128]; pk32[11, 0:128] = inputs['gb4'][128:]
    pk32[12, 0:128] = inputs['fb1'][:128]; pk32[13, 0:128] = inputs['fb1'][128:]
    pk32[14, 0:128] = inputs['fb2'][:128]; pk32[15, 0:128] = inputs['fb2'][128:]
    pk32[0:32, 128:160] = np.eye(32, dtype=np.float32)
    pk32[0, 160] = float(np.asarray(inputs['fb3']).reshape(-1)[0])
    return pk, pk8, pk32


def _build(num_devices, loop_n=0, section="all", rn_fp8=True):
    import concourse.bacc as bacc
    import concourse.mybir as mybir
    import concourse.tile as tile

    f16 = mybir.dt.float16
    f32 = mybir.dt.float32
    f8 = mybir.dt.float8e4
    AF = mybir.ActivationFunctionType
    OP = mybir.AluOpType
    DR = mybir.MatmulPerfMode.DoubleRow

    NB = B // 4                  # conv1/conv2 4-sample blocks
    run_conv = section in ("all", "conv")
    run_rn = section in ("all", "rn")

    nc = bacc.Bacc(None, target_bir_lowering=False, num_devices=num_devices)

    # imc = host-side im2col of the (permuted) image batch:
    # imc[j*48+dy*12+ic, s, oy, 1-j+u] = img[s, ic, 2*oy+dy-1, u], zero-padded
    imc = nc.dram_tensor("imc", [96, B, 32, 66], f16, kind="ExternalInput")
    pk16 = nc.dram_tensor("pk16", [128, P16_W], f16, kind="ExternalInput")
    pk8d = nc.dram_tensor("pk8", [128, P8_W], f8, kind="ExternalInput")
    pk32 = nc.dram_tensor("pk32", [32, P32_W], f32, kind="ExternalInput")
    out = nc.dram_tensor("out", [1, BPC], f32, kind="ExternalOutput")

    with tile.TileContext(nc) as tc, ExitStack() as ctx:
        cw = ctx.enter_context(tc.tile_pool(name="cw", bufs=1))
        st = ctx.enter_context(tc.tile_pool(name="st", bufs=1))
        sm = ctx.enter_context(tc.tile_pool(name="sm", bufs=2))
        trash = ctx.enter_context(tc.tile_pool(name="trash", bufs=4))
        big = ctx.enter_context(tc.tile_pool(name="big", bufs=3, space="PSUM"))
        hold = ctx.enter_context(tc.tile_pool(name="hold", bufs=2, space="PSUM"))

        # ---- persistent tile allocations (all DMAs/memsets live in body) ----
        PK = cw.tile([128, P16_W], f16, name="PK")
        PK8 = cw.tile([128, P8_W], f8, name="PK8")
        P32 = cw.tile([32, P32_W], f32, name="P32")

        def pk_at(nm, o0, width, rows=128, r0=0):
            o = _P16_OFF[nm] + o0
            return PK[r0:r0 + rows, o:o + width]

        def pk8_dual(nm, kc, mc):
            o = _P8_OFF[nm] + (kc * 2 + mc) * 256
            return PK8[:, o:o + 256].rearrange("p (pl m) -> p pl m", pl=2)

        def gsel8_at(tt):
            o = _P8_OFF["gsel8"] + tt * 512
            return PK8[:, o:o + 512].rearrange(
                "p (o n) -> p o n", o=1).broadcast_to([128, 2, 512])

        def bcast2(ap):
            # [p, n] -> [p, 2, n] stride-0 plane for dual-fp8 DoubleRow
            return ap.rearrange("p (o n) -> p o n", o=1).broadcast_to(
                [ap.partition_size(), 2, ap.free_size()])

        csb = P32[0:20, 0:128]
        id_s = P32[0:32, 128:160]
        fb3_s = P32[0:1, 160:161]
        asb = P32[0:BPC, 161:161 + AUX]

        imfs = [st.tile([96, 4, 32, 66], f16, name=f"imf{t}") for t in range(2)]
        H1s = [st.tile([128, 4, 34, 34], f16, name=f"H1{t}") for t in range(2)]
        H2 = st.tile([128, B, 18, 18], f16, name="H2")
        C2R = st.tile([128, B // 2, 512], f16, name="C2R")  # raw conv2 (s,y,x)
        ENC = st.tile([128, 2, BPC, 64], f16, name="ENC")
        excl = st.tile([11, 64], f16, name="excl")
        rn_dt = f8 if rn_fp8 else f16
        bufs = [st.tile([128, 2, NUM * NUM], rn_dt, name=f"rnbuf{i}")
                for i in range(3)]
        POOL = st.tile([128, BPC, 2], f32, name="POOL")

        dma_engines = [nc.sync, nc.gpsimd, nc.scalar]

        def bn_finalize_mv(mv, g_col, b_col):
            # mv = [128, 2] (mean, biased var) from bn_aggr
            std = sm.tile([128, 1], f32, tag="bns")
            epsb = sm.tile([128, 1], f32, tag="bne")
            nc.vector.memset(epsb, EPS)
            nc.scalar.activation(out=std, in_=mv[:, 1:2], func=AF.Sqrt,
                                 bias=epsb[:, 0:1])
            rstd = sm.tile([128, 1], f32, tag="bnr")
            nc.vector.reciprocal(rstd, std)
            a_c = sm.tile([128, 1], f32, tag="bna")
            nc.vector.tensor_tensor(out=a_c, in0=rstd, in1=g_col, op=OP.mult)
            d_c = sm.tile([128, 1], f32, tag="bnd")
            nc.vector.tensor_tensor(out=d_c, in0=mv[:, 0:1], in1=a_c,
                                    op=OP.mult)
            nc.vector.tensor_tensor(out=d_c, in0=b_col, in1=d_c, op=OP.subtract)
            return a_c, d_c

        def conv_part(col):
            # ---- conv1 + conv2 pipeline over 4-sample blocks ----
            s2ch = sm.tile([128, 2 * NB, 6], f32, tag="s2ch")
            for b in range(NB):
                imf = imfs[b % 2]
                dma_engines[b % 3].dma_start(
                    out=imf, in_=imc[:, 4 * b:4 * b + 4, :, :])
                imv = imf.rearrange("p s y (v c) -> p s y v c", c=2)

                H1 = H1s[b % 2]
                for s in range(4):
                    pt = big.tile([128, 1024], f32, tag="mm")
                    ptv = pt.rearrange("p (y x) -> p y x", x=32)
                    for m in range(2):
                        for oyh in range(2):
                            nc.tensor.matmul(
                                ptv[:, oyh * 16:(oyh + 1) * 16, :],
                                pk_at("c1w", m * 128, 128, rows=96),
                                imv[:, s, oyh * 16:(oyh + 1) * 16, m:m + 32, 0],
                                start=(m == 0), stop=(m == 1),
                            )
                    # j=0 half at x+1 (ACT/DVE alternating)
                    if s % 2 == 0:
                        nc.scalar.activation(out=H1[0:64, s, 1:33, 1:33],
                                             in_=ptv[0:64],
                                             func=AF.Prelu, alpha=0.2)
                    else:
                        tmp = trash.tile([64, 1024], f16, tag="lr")
                        nc.vector.tensor_scalar(out=tmp, in0=ptv[0:64],
                                                scalar1=0.2, scalar2=None,
                                                op0=OP.mult)
                        nc.vector.tensor_tensor(
                            out=H1[0:64, s, 1:33, 1:33],
                            in0=ptv[0:64],
                            in1=tmp.rearrange("p (y x) -> p y x", x=32),
                            op=OP.max)
                # j=1 (x-shifted) copy for conv2's K=128 packing: one DMA
                (nc.sync if b % 2 == 0 else nc.gpsimd).dma_start(
                    out=H1[64:128, :, :, 0:33], in_=H1[0:64, :, :, 1:34])

                # conv2 on this block (two 2-sample tiles)
                H1v = H1.rearrange("p s (r a) (v c) -> p s r a v c", a=2, c=2)
                for sb in range(2):
                    b2 = b * 2 + sb
                    pt2 = hold.tile([128, 512], f32, tag="hold")
                    k = 0
                    for dy in range(4):
                        for m in range(2):
                            nc.tensor.matmul(
                                pt2,
                                pk_at("c2w", (dy * 2 + m) * 128, 128),
                                H1v[:, sb * 2:sb * 2 + 2,
                                    dy // 2:dy // 2 + 16, dy % 2,
                                    m:m + 16, 0],
                                start=(k == 0), stop=(k == 7),
                            )
                            k += 1
                    nc.vector.bn_stats(out=s2ch[:, b2, :], in_=pt2)
                    if b2 % 2 == 0:
                        nc.scalar.activation(out=C2R[:, b2, :], in_=pt2,
                                             func=AF.Identity)
                    else:
                        nc.vector.tensor_copy(C2R[:, b2, :], pt2)

            # ---- BN2 + lrelu -> H2 ----
            mv2 = sm.tile([128, 2], f32, tag="mv2")
            nc.vector.bn_aggr(out=mv2, in_=s2ch.rearrange("p c s -> p (c s)"))
            a2, d2 = bn_finalize_mv(mv2, col(0), col(1))
            C2Rv = C2R.rearrange("p nb (s y x) -> p (nb s) y x", s=2, x=16)
            for h in range(2):
                nc.scalar.activation(
                    out=H2[:, h * 16:(h + 1) * 16, 1:17, 1:17],
                    in_=C2Rv[:, h * 16:(h + 1) * 16],
                    func=AF.Prelu, bias=d2[:, 0:1], scale=a2[:, 0:1], alpha=0.2)

            # ---- conv3: PSUM-resident per mc; evict only stats + own 4 ----
            H2v = H2.rearrange("p s (r a) (v c) -> p s r a v c", a=2, c=2)
            s3ch = sm.tile([128, 2, 4, 6], f32, tag="s3ch")
            for mc in range(2):
                tiles = []
                for half in range(2):
                    pt3 = big.tile([128, 1024], f32, tag="mm")
                    tiles.append(pt3)
                    k = 0
                    for dy in range(4):
                        for dx in range(4):
                            for q in range(2):
                                s0 = half * 16 + q * 8
                                nc.tensor.matmul(
                                    pt3[:, q * 512:(q + 1) * 512],
                                    pk_at("c3w",
                                          ((dy * 4 + dx) * 2 + mc) * 128, 128),
                                    H2v[:, s0:s0 + 8, dy // 2:dy // 2 + 8,
                                        dy % 2, dx // 2:dx // 2 + 8, dx % 2],
                                    start=(k == 0), stop=(k == 15),
                                )
                            k += 1
                    for q in range(2):
                        nc.vector.bn_stats(
                            out=s3ch[:, mc, 2 * half + q, :],
                            in_=pt3[:, q * 512:(q + 1) * 512])
                mv3 = sm.tile([128, 2], f32, tag="mv3")
                nc.vector.bn_aggr(out=mv3,
                                  in_=s3ch[:, mc].rearrange("p c s -> p (c s)"))
                a3, d3 = bn_finalize_mv(mv3, col(2 + mc), col(4 + mc))
                nc.scalar.activation(
                    out=ENC[:, mc, :, :],
                    in_=tiles[0][:, 0:BPC * 64].rearrange(
                        "p (s x) -> p s x", s=BPC),
                    func=AF.Prelu, bias=d3[:, 0:1], scale=a3[:, 0:1], alpha=0.2)

        def rn_part(col, relu_bias, AUXT):
            for s in range(BPC):
                nc.vector.tensor_copy(excl[0:AUX, :],
                                      AUXT[:, s:s + 1].to_broadcast((AUX, 64)))
                acp = hold.tile([128, 512], f32, tag="hold")
                nc.tensor.matmul(acp[0:64, 0:256], ENC[:, 0, s, :],
                                 pk_at("w1t", 0, 256), start=True, stop=False)
                nc.tensor.matmul(acp[0:64, 0:256], ENC[:, 1, s, :],
                                 pk_at("w1t", 256, 256), start=False, stop=False)
                nc.tensor.matmul(acp[0:64, 0:256], pk_at("aro", 0, 64, rows=2),
                                 pk_at("exA", 0, 256, rows=2),
                                 start=False, stop=True)
                nc.tensor.matmul(acp[64:128, 0:256], ENC[:, 0, s, :],
                                 pk_at("w1b", 0, 256), start=True, stop=False)
                nc.tensor.matmul(acp[64:128, 0:256], ENC[:, 1, s, :],
                                 pk_at("w1b", 256, 256), start=False, stop=False)
                nc.tensor.matmul(acp[64:128, 0:256], excl,
                                 pk_at("exC", 0, 256, rows=11),
                                 start=False, stop=True)
                if rn_fp8:
                    ac8 = sm.tile([128, 2, 256], f8, tag="ac8")
                    nc.scalar.copy(ac8[:, 0, :], acp[:, 0:256])
                    nc.vector.scalar_tensor_tensor(
                        out=ac8[:, 1, :], in0=acp[:, 0:256], scalar=1.0,
                        in1=ac8[:, 0, :], op0=OP.mult, op1=OP.subtract)
                else:
                    ac = sm.tile([128, 256], f16, tag="ac")
                    nc.scalar.copy(ac, acp[:, 0:256])

                b0, b1, b2 = bufs
                # L1: S.T = AC.T @ G, relu -> b0 (mc outer: stationary reuse)
                for mc in range(2):
                    for t2 in range(4):
                        pt = big.tile([128, 1024], f32, tag="mm")
                        for h in range(2):
                            tt = t2 * 2 + h
                            if rn_fp8:
                                nc.tensor.matmul(
                                    pt[:, h * 512:(h + 1) * 512],
                                    ac8[:, :, mc * 128:(mc + 1) * 128],
                                    gsel8_at(tt),
                                    start=True, stop=True, perf_mode=DR)
                            else:
                                nc.tensor.matmul(
                                    pt[:, h * 512:(h + 1) * 512],
                                    ac[:, mc * 128:(mc + 1) * 128],
                                    pk_at("gsel", tt * 512, 512),
                                    start=True, stop=True)
                        relu_bias((0, 1, 0, 1, 0, 1, 0, 1)[mc * 4 + t2],
                                  b0[:, mc, t2 * 1024:(t2 + 1) * 1024], pt,
                                  None)
                # L2..L4 dual-fp8 DoubleRow (or kc-outer fp16)
                for li, (wnm, bci, src, dst) in enumerate((
                        ("gw2", 6, b0, b1), ("gw3", 8, b1, b2),
                        ("gw4", 10, b2, None))):
                    if li == 2:
                        p4 = sm.tile([128, 2, 4], f32, tag="p4")
                    for mc in range(2):
                        for t2 in range(4):
                            pt = big.tile([128, 1024], f32, tag="mm")
                            for h in range(2):
                                tt = t2 * 2 + h
                                for kc in range(2):
                                    if rn_fp8:
                                        nc.tensor.matmul(
                                            pt[:, h * 512:(h + 1) * 512],
                                            pk8_dual(wnm, kc, mc),
                                            bcast2(src[:, kc,
                                                       tt * 512:(tt + 1) * 512]),
                                            start=(kc == 0), stop=(kc == 1),
                                            perf_mode=DR)
                                    else:
                                        nc.tensor.matmul(
                                            pt[:, h * 512:(h + 1) * 512],
                                            pk_at(wnm,
                                                  (kc * 2 + mc) * 128, 128),
                                            src[:, kc,
                                                tt * 512:(tt + 1) * 512],
                                            start=(kc == 0), stop=(kc == 1))
                            sc = INV_SG if rn_fp8 else None
                            if li < 2:
                                relu_bias((0, 1, 0, 1, 0, 0, 1, 0)[mc * 4 + t2],
                                          dst[:, mc,
                                              t2 * 1024:(t2 + 1) * 1024],
                                          pt, col(bci + mc), scale=sc)
                            else:
                                tb = trash.tile([128, 1024], f16, tag="tr4")
                                relu_bias((0, 1, 0, 1, 0, 1, 0, 1)[mc * 4 + t2],
                                          tb, pt, col(bci + mc),
                                          accum=p4[:, mc, t2:t2 + 1], scale=sc)
                nc.vector.tensor_reduce(out=POOL[:, s, :], in_=p4,
                                        axis=mybir.AxisListType.X, op=OP.add)

            # ---- f MLP ----
            POOLh = st.tile([128, BPC, 2], f16, tag="poolh")
            nc.vector.tensor_copy(POOLh, POOL)
            F1 = st.tile([128, 2, BPC], f16, tag="f1")
            F2 = st.tile([128, 2, BPC], f16, tag="f2")
            for (wnm, bci, src, dst) in (("fw1", 12, POOLh, F1),
                                         ("fw2", 14, F1, F2)):
                for mc in range(2):
                    pt = hold.tile([128, 512], f32, tag="hold")
                    if src is POOLh:
                        r0, r1 = src[:, :, 0], src[:, :, 1]
                    else:
                        r0, r1 = src[:, 0, :], src[:, 1, :]
                    nc.tensor.matmul(pt[:, 0:BPC], pk_at(wnm, mc * 128, 128),
                                     r0, start=True, stop=False)
                    nc.tensor.matmul(pt[:, 0:BPC],
                                     pk_at(wnm, (2 + mc) * 128, 128),
                                     r1, start=False, stop=True)
                    relu_bias(0, dst[:, mc, :], pt[:, 0:BPC], col(bci + mc))
            pt = hold.tile([128, 512], f32, tag="hold")
            nc.tensor.matmul(pt[0:1, 0:BPC], pk_at("fw3", 0, 1), F2[:, 0, :],
                             start=True, stop=False)
            nc.tensor.matmul(pt[0:1, 0:BPC], pk_at("fw3", 1, 1), F2[:, 1, :],
                             start=False, stop=True)
            osb = sm.tile([1, BPC], f32, tag="osb")
            nc.scalar.activation(out=osb, in_=pt[0:1, 0:BPC], func=AF.Identity,
                                 bias=fb3_s[0:1, 0:1])
            nc.sync.dma_start(out=out[:, :], in_=osb)

        def body():
            nc.sync.dma_start(out=PK, in_=pk16.ap())
            nc.scalar.dma_start(out=P32, in_=pk32.ap())
            nc.gpsimd.dma_start(out=PK8, in_=pk8d.ap())
            nc.sync.dma_start(out=excl[10:11, :],
                              in_=pk_at("aro", 0, 64, rows=1))
            for H1 in H1s:
                nc.vector.memset(H1[:, :, 0:1, :], 0.0)
                nc.vector.memset(H1[:, :, 33:34, :], 0.0)
                nc.vector.memset(H1[:, :, :, 0:1], 0.0)
                nc.vector.memset(H1[:, :, :, 33:34], 0.0)
            nc.vector.memset(H2[:, :, 0:1, :], 0.0)
            nc.vector.memset(H2[:, :, 17:18, :], 0.0)
            nc.vector.memset(H2[:, :, :, 0:1], 0.0)
            nc.vector.memset(H2[:, :, :, 17:18], 0.0)

            # consts transpose -> CONST_T [128, 20]
            CONST_T = st.tile([128, 20], f32, tag="constT")
            ptc = hold.tile([128, 512], f32, tag="hold")
            nc.tensor.transpose(ptc[:, 0:20], csb, id_s[0:20, 0:20])
            nc.vector.tensor_copy(CONST_T, ptc[:, 0:20])
            AUXT = st.tile([AUX, BPC], f16, tag="auxT")
            pta = hold.tile([128, 512], f32, tag="hold")
            nc.tensor.transpose(pta[0:AUX, 0:BPC], asb, id_s[0:BPC, 0:BPC])
            nc.vector.tensor_copy(AUXT, pta[0:AUX, 0:BPC])

            def col(i):
                return CONST_T[:, i:i + 1]

            zcol = sm.tile([128, 1], f32, tag="zcol")
            nc.vector.memset(zcol, 0.0)

            def relu_bias(eng, dst, psum, bias_col, accum=None, scale=None):
                # eng: 0=ACT, 1=DVE. relu(psum*scale + bias); the DVE path
                # with scale drops bias (asserted zero host-side).
                if eng is True:
                    eng = 0
                elif eng is False:
                    eng = 1
                if eng == 0:
                    nc.scalar.activation(
                        out=dst, in_=psum, func=AF.Relu,
                        bias=bias_col if bias_col is not None else 0.0,
                        scale=scale if scale is not None else 1.0,
                        accum_out=accum)
                elif accum is not None:
                    if scale is not None:
                        nc.vector.scalar_tensor_tensor(
                            out=dst, in0=psum, scalar=scale,
                            in1=zcol[:, 0:1].to_broadcast(
                                (psum.partition_size(), psum.free_size())),
                            op0=OP.mult, op1=OP.max, accum_out=accum)
                    else:
                        nc.vector.scalar_tensor_tensor(
                            out=dst, in0=psum,
                            scalar=bias_col if bias_col is not None else 0.0,
                            in1=zcol[:, 0:1].to_broadcast(
                                (psum.partition_size(), psum.free_size())),
                            op0=OP.add, op1=OP.max, accum_out=accum)
                else:
                    if scale is not None:
                        nc.vector.tensor_scalar(
                            out=dst, in0=psum, scalar1=scale,
                            scalar2=0.0, op0=OP.mult, op1=OP.max)
                    else:
                        nc.vector.tensor_scalar(
                            out=dst, in0=psum,
                            scalar1=bias_col if bias_col is not None else 0.0,
                            scalar2=0.0, op0=OP.add, op1=OP.max)

            if run_conv:
                conv_part(col)
            else:
                nc.vector.memset(ENC, 0.02)
            if run_rn:
                rn_part(col, relu_bias, AUXT)
            else:
                osb = sm.tile([1, BPC], f32, tag="osb")
                nc.vector.tensor_copy(osb, ENC[0:1, 0, :, 0])
                nc.sync.dma_start(out=out[:, :], in_=osb)

        if loop_n > 0:
            with tc.For_i(0, loop_n, 1):
                body()
        else:
            body()

    nc.compile()
    return nc


def _host_im2col(img):
    """img [B, 12, 64, 64] fp16 -> imc [96, B, 32, 66] fp16.
    imc[dy*12+ic, s, oy, 1+u] = img[s, ic, 2*oy+dy-1, u] (j=0 rows),
    rows 48:96 = rows 0:48 shifted one column left (j=1), zero-padded."""
    f16 = np.float16
    imc = np.zeros((96, B, 32, 66), f16)
    oy = np.arange(32)
    for dy in range(4):
        h = 2 * oy + dy - 1
        ok = (h >= 0) & (h < 64)
        # [s, ic, oy_ok, u] -> [ic, s, oy_ok, u]
        imc[dy * 12:(dy + 1) * 12, :, ok, 1:65] = \
            img[:, :, h[ok], :].transpose(1, 0, 2, 3)
    imc[48:96, :, :, 0:65] = imc[0:48, :, :, 1:66]
    return imc


def _in_maps(inputs):
    pk, pk8, pk32 = _host_pack(inputs)
    img = np.asarray(inputs['input_image'], dtype=np.float16)
    aux = np.asarray(inputs['inputg_aux_v'], dtype=np.float32)
    imc0 = _host_im2col(img)
    in_maps = []
    for c in range(NCORES):
        own = list(range(c * BPC, (c + 1) * BPC))
        rest = [i for i in range(B) if i not in own]
        perm = own + rest
        p32c = pk32.copy()
        p32c[0:BPC, 161:161 + AUX] = aux[own]
        in_maps.append({'imc': np.ascontiguousarray(imc0[:, perm]),
                        'pk16': pk, 'pk8': pk8, 'pk32': p32c})
    return in_maps


def kernel(**inputs):
    from concourse.bass_utils import run_bass_kernel_spmd

    key = ("nc3", NCORES)
    if key not in _CACHE:
        _CACHE[key] = _build(NCORES)
    nc = _CACHE[key]

    in_maps = _in_maps(inputs)
    res = run_bass_kernel_spmd(nc, in_maps, core_ids=list(range(NCORES)))
    kernel.last_result = res
    outs = [res.results[c]["out"].reshape(BPC, 1) for c in range(NCORES)]
    return np.concatenate(outs, axis=0).astype(np.float32)


# revision 32
# speedup vs baseline: 1.1399x; 1.1399x over previous
"""Trainium2 Bass kernel v3 for nn_DCGAN_C (DCGAN conv backbone + relation net).

Key changes vs v2 (327.9us -> target ~150us):
- RN layers use dual-plane fp8 DoubleRow matmuls: each fp16 weight W is split
  host-side into W_hi = fp8(W*S) and W_lo = fp8(W*S - W_hi) and packed as the
  two DoubleRow K-planes; the moving operand is broadcast (stride-0 plane), so
  out = (W_hi + W_lo) @ x at 0.5 cyc/row with ~0.1% effective weight error.
  The PSUM carries factor S; evictions apply 1/S (ACT scale= / DVE mult-max).
  The L1 A+C expansion uses the same trick with an on-chip hi/lo split of AC.
- conv1's second (j=1, x-shifted) H1 copy is made by one SBUF->SBUF DMA per
  block instead of per-sample engine evictions.
- BN statistics via DVE bn_stats/bn_aggr (single pass, no Square pass).
- conv3 output stays resident in PSUM per mc-half; only channel stats (DVE)
  and the core's own 4 samples (ENC) are ever evicted.
- No collectives: every core runs the full-batch conv backbone (exact BN
  batch stats), RN only on its own 4 samples. Pure SPMD.
- `loop_n` wraps the body in tc.For_i for steady-state benchmarking.
"""
import os
import numpy as np
from contextlib import ExitStack

NCORES = 8
B = 32           # full batch (replicated conv backbone)
BPC = 4          # samples per core for the relation network
NUM = 64
AUX = 10
EPS = 1e-5
S_G = 32.0       # RN weight pre-scale for the fp8 hi/lo split
INV_SG = 1.0 / S_G
S_IMG = 1.0      # conv runs in fp16: neutral scales
S_C1 = 1.0
S_C2 = 1.0
S_C3 = 1.0
S_H1 = 1.0
S_H2 = 1.0
SC1EV = S_H1 / (S_IMG * S_C1)       # conv1 eviction scale
EPS2 = EPS * (S_H1 * S_C2) ** 2     # eps in scaled-psum units
EPS3 = EPS * (S_H2 * S_C3) ** 2

_CACHE = {}

# fp16 const-pack layout: (name, rows, width)
_P16 = [
    ("c1w", 96, 2 * 64),         # [j*48+dy*12+ic][m*64 + oc]
    ("c2w", 128, 8 * 128),       # [j*64+ic][(dy*2+m)*128 + oc]
    ("c3w", 128, 32 * 128),      # [ic][((dy*4+dx)*2+mc)*128 + oc']
    ("w1t", 128, 2 * 256),       # [p][kc*256 + c]
    ("w1b", 128, 2 * 256),
    ("exA", 2, 256),
    ("exC", 11, 256),
    ("aro", 2, 64),              # row0 arange, row1 ones
    ("fw1", 128, 4 * 128),
    ("fw2", 128, 4 * 128),
    ("fw3", 128, 2),
]
# fp8e4 pack: dual hi/lo (pl-major within each 2*M slice)
# c1w8: per m in 2: [j*48+dy*12+ic][m*128 + pl*64 + oc]        (M=64)
# c2w8: per (dy,m): [j*64+ic][(dy*2+m)*256 + pl*128 + oc]      (M=128)
# c3w8: per (dy,dx,mc): [ic][((dy*4+dx)*2+mc)*256 + pl*128+oc] (M=128)
# gw*:  per (kc,mc):  [(kc*2+mc)*256 + pl*128 + m]
_P8 = [("gw2", 1024), ("gw3", 1024), ("gw4", 1024), ("gsel8", NUM * NUM)]
_P8_OFF = {}
_off8 = 0
for _nm, _w in _P8:
    _P8_OFF[_nm] = _off8
    _off8 += _w
P8_W = _off8
_P16_OFF = {}
_off = 0
for _nm, _r, _w in _P16:
    _P16_OFF[_nm] = _off
    _off += _w
P16_W = _off
# fp32 pack [32, 204]: consts rows 0-19 cols 0-128; ident [0:32,128:160];
# fb3 [0,160]; aux [0:BPC, 161:171]
P32_W = 204


def _dual8(W, scale):
    """Split W*scale into fp8 hi + lo planes (returned as fp8 arrays)."""
    import ml_dtypes
    f8 = ml_dtypes.float8_e4m3
    hi = (W * scale).astype(f8)
    lo = (W * scale - hi.astype(np.float32)).astype(f8)
    return hi, lo


def _host_pack(inputs):
    f16 = np.float16
    w1 = inputs['conv1_w'].astype(np.float32)
    w2 = inputs['conv2_w'].astype(np.float32)
    w3 = inputs['conv3_w'].astype(np.float32)
    gW1 = inputs['gW1'].astype(np.float32)
    gb1 = inputs['gb1'].astype(np.float32)
    # the dual-fp8 DVE eviction path drops layer biases (they are zero for
    # this model); assert that stays true.
    for nm in ('gb2', 'gb3', 'gb4'):
        assert np.abs(np.asarray(inputs[nm])).max() == 0.0, nm

    pk = np.zeros((128, P16_W), f16)

    def put(nm, arr):
        o = _P16_OFF[nm]
        r, w = arr.shape
        pk[0:r, o:o + w] = arr.astype(f16)

    c1 = np.zeros((96, 128), np.float32)
    for j in range(2):
        for dy in range(4):
            for m in range(2):
                c1[j * 48 + dy * 12:j * 48 + dy * 12 + 12,
                   m * 64:(m + 1) * 64] = w1[:, :, dy, 2 * m + j].T
    put("c1w", c1)

    c2 = np.zeros((128, 8 * 128), np.float32)
    for dy in range(4):
        for m in range(2):
            o = (dy * 2 + m) * 128
            for j in range(2):
                c2[j * 64:(j + 1) * 64, o:o + 128] = w2[:, :, dy, 2 * m + j].T
    put("c2w", c2)

    c3 = np.zeros((128, 32 * 128), np.float32)
    for dy in range(4):
        for dx in range(4):
            for mc in range(2):
                o = ((dy * 4 + dx) * 2 + mc) * 128
                c3[:, o:o + 128] = w3[mc * 128:(mc + 1) * 128, :, dy, dx].T
    put("c3w", c3 * S_C3)

    w1t = np.zeros((128, 512), np.float32)
    w1b = np.zeros((128, 512), np.float32)
    for kc in range(2):
        w1t[:, kc * 256:(kc + 1) * 256] = gW1[kc * 128:(kc + 1) * 128]
        w1b[:, kc * 256:(kc + 1) * 256] = gW1[257 + kc * 128:257 + (kc + 1) * 128]
    put("w1t", w1t)
    put("w1b", w1b)
    put("exA", np.stack([gW1[256], gb1], 0))
    put("exC", np.concatenate([gW1[513:523], gW1[523:524]], 0))
    put("aro", np.stack([np.arange(64, dtype=np.float32),
                         np.ones(64, np.float32)], 0))

    def sq(nm, W):
        o = np.zeros((128, 512), np.float32)
        for kc in range(2):
            for mc in range(2):
                o[:, (kc * 2 + mc) * 128:(kc * 2 + mc + 1) * 128] = \
                    W[kc * 128:(kc + 1) * 128, mc * 128:(mc + 1) * 128]
        put(nm, o)

    sq("fw1", inputs['fW1']); sq("fw2", inputs['fW2'])
    f3 = np.zeros((128, 2), np.float32)
    for kc in range(2):
        f3[:, kc] = inputs['fW3'][kc * 128:(kc + 1) * 128, 0]
    put("fw3", f3)

    G = np.zeros((128, NUM * NUM), np.float32)
    ii, jj = np.meshgrid(np.arange(NUM), np.arange(NUM), indexing='ij')
    p = (ii * NUM + jj).ravel()
    G[jj.ravel(), p] = 1.0
    G[64 + ii.ravel(), p] = 1.0

    import ml_dtypes
    pk8 = np.zeros((128, P8_W), ml_dtypes.float8_e4m3)

    def put8_dual(nm, grp, W, scale):
        hi, lo = _dual8(W, scale)
        r, m = hi.shape
        c0 = _P8_OFF[nm] + grp * 2 * m
        pk8[0:r, c0:c0 + m] = hi
        pk8[0:r, c0 + m:c0 + 2 * m] = lo

    for nm in ("gw2", "gw3", "gw4"):
        W = np.asarray(inputs['gW' + nm[-1]], np.float32)
        for kc in range(2):
            for mc in range(2):
                put8_dual(nm, kc * 2 + mc,
                          W[kc * 128:(kc + 1) * 128, mc * 128:(mc + 1) * 128],
                          S_G)
    pk8[:, _P8_OFF["gsel8"]:_P8_OFF["gsel8"] + NUM * NUM] = \
        G.astype(ml_dtypes.float8_e4m3)

    pk32 = np.zeros((32, P32_W), np.float32)
    pk32[0, 0:128] = S_H2 * np.asarray(inputs['bn2_g'], np.float32)
    pk32[1, 0:128] = S_H2 * np.asarray(inputs['bn2_b'], np.float32)
    pk32[2, 0:128] = inputs['bn3_g'][:128]; pk32[3, 0:128] = inputs['bn3_g'][128:]
    pk32[4, 0:128] = inputs['bn3_b'][:128]; pk32[5, 0:128] = inputs['bn3_b'][128:]
    pk32[6, 0:128] = inputs['gb2'][:128]; pk32[7, 0:128] = inputs['gb2'][128:]
    pk32[8, 0:128] = inputs['gb3'][:128]; pk32[9, 0:128] = inputs['gb3'][128:]
    pk32[10, 0:128] = inputs['gb4'][:128]; pk32[11, 0:128] = inputs['gb4'][128:]
    pk32[12, 0:128] = inputs['fb1'][:128]; pk32[13, 0:128] = inputs['fb1'][128:]
    pk32[14, 0:128] = inputs['fb2'][:128]; pk32[15, 0:128] = inputs['fb2'][128:]
    pk32[0:32, 128:160] = np.eye(32, dtype=np.float32)
    pk32[0, 160] = float(np.asarray(inputs['fb3']).reshape(-1)[0])
    return pk, pk8, pk32


def _build(num_devices, loop_n=0, section="all", rn_fp8=True):
    import concourse.bacc as bacc
    import concourse.mybir as mybir
    import concourse.tile as tile

    f16 = mybir.dt.float16
    f32 = mybir.dt.float32
    f8 = mybir.dt.float8e4
    AF = mybir.ActivationFunctionType
    OP = mybir.AluOpType
    DR = mybir.MatmulPerfMode.DoubleRow

    NB = B // 4                  # conv1/conv2 4-sample blocks
    run_conv = section in ("all", "conv")
    run_rn = section in ("all", "rn")

    nc = bacc.Bacc(None, target_bir_lowering=False, num_devices=num_devices)

    # imc = host-side im2col of the (permuted) image batch:
    # imc[j*48+dy*12+ic, s, oy, 1-j+u] = img[s, ic, 2*oy+dy-1, u], zero-padded
    imc = nc.dram_tensor("imc", [96, B, 32, 66], f16, kind="ExternalInput")
    pk16 = nc.dram_tensor("pk16", [128, P16_W], f16, kind="ExternalInput")
    pk8d = nc.dram_tensor("pk8", [128, P8_W], f8, kind="ExternalInput")
    pk32 = nc.dram_tensor("pk32", [32, P32_W], f32, kind="ExternalInput")
    out = nc.dram_tensor("out", [1, BPC], f32, kind="ExternalOutput")

    with tile.TileContext(nc) as tc, ExitStack() as ctx:
        cw = ctx.enter_context(tc.tile_pool(name="cw", bufs=1))
        st = ctx.enter_context(tc.tile_pool(name="st", bufs=1))
        sm = ctx.enter_context(tc.tile_pool(name="sm", bufs=2))
        trash = ctx.enter_context(tc.tile_pool(name="trash", bufs=4))
        big = ctx.enter_context(tc.tile_pool(name="big", bufs=3, space="PSUM"))
        hold = ctx.enter_context(tc.tile_pool(name="hold", bufs=2, space="PSUM"))

        # ---- persistent tile allocations (all DMAs/memsets live in body) ----
        PK = cw.tile([128, P16_W], f16, name="PK")
        PK8 = cw.tile([128, P8_W], f8, name="PK8")
        P32 = cw.tile([32, P32_W], f32, name="P32")

        def pk_at(nm, o0, width, rows=128, r0=0):
            o = _P16_OFF[nm] + o0
            return PK[r0:r0 + rows, o:o + width]

        def pk8_dual(nm, kc, mc):
            o = _P8_OFF[nm] + (kc * 2 + mc) * 256
            return PK8[:, o:o + 256].rearrange("p (pl m) -> p pl m", pl=2)

        def gsel8_at(tt):
            o = _P8_OFF["gsel8"] + tt * 512
            return PK8[:, o:o + 512].rearrange(
                "p (o n) -> p o n", o=1).broadcast_to([128, 2, 512])

        def bcast2(ap):
            # [p, n] -> [p, 2, n] stride-0 plane for dual-fp8 DoubleRow
            return ap.rearrange("p (o n) -> p o n", o=1).broadcast_to(
                [ap.partition_size(), 2, ap.free_size()])

        csb = P32[0:20, 0:128]
        id_s = P32[0:32, 128:160]
        fb3_s = P32[0:1, 160:161]
        asb = P32[0:BPC, 161:161 + AUX]

        imfs = [st.tile([96, 4, 32, 66], f16, name=f"imf{t}") for t in range(2)]
        H1s = [st.tile([128, 4, 34, 34], f16, name=f"H1{t}") for t in range(2)]
        H2 = st.tile([128, B, 18, 18], f16, name="H2")
        C2R = st.tile([128, B // 2, 512], f16, name="C2R")  # raw conv2 (s,y,x)
        ENC = st.tile([128, 2, BPC, 64], f16, name="ENC")
        excl = st.tile([11, 64], f16, name="excl")
        rn_dt = f8 if rn_fp8 else f16
        bufs = [st.tile([128, 2, NUM * NUM], rn_dt, name=f"rnbuf{i}")
                for i in range(3)]
        POOL = st.tile([128, BPC, 2], f32, name="POOL")

        dma_engines = [nc.sync, nc.gpsimd]

        def pk8_conv(nm, grp, rows=128, m=128):
            o = _P8_OFF[nm] + grp * 2 * m
            return PK8[0:rows, o:o + 2 * m].rearrange(
                "p (pl m) -> p pl m", pl=2)

        def bn_finalize_mv(mv, g_col, b_col, eps_v):
            # mv = [128, 2] (mean, biased var) from bn_aggr
            std = sm.tile([128, 1], f32, tag="bns")
            epsb = sm.tile([128, 1], f32, tag="bne")
            nc.vector.memset(epsb, eps_v)
            nc.scalar.activation(out=std, in_=mv[:, 1:2], func=AF.Sqrt,
                                 bias=epsb[:, 0:1])
            rstd = sm.tile([128, 1], f32, tag="bnr")
            nc.vector.reciprocal(rstd, std)
            a_c = sm.tile([128, 1], f32, tag="bna")
            nc.vector.tensor_tensor(out=a_c, in0=rstd, in1=g_col, op=OP.mult)
            d_c = sm.tile([128, 1], f32, tag="bnd")
            nc.vector.tensor_tensor(out=d_c, in0=mv[:, 0:1], in1=a_c,
                                    op=OP.mult)
            nc.vector.tensor_tensor(out=d_c, in0=b_col, in1=d_c, op=OP.subtract)
            return a_c, d_c

        def conv_part(col):
            # ---- conv1 + conv2 pipeline over 4-sample blocks ----
            s2ch = sm.tile([128, 2 * NB, 6], f32, tag="s2ch")
            for b in range(NB):
                imf = imfs[b % 2]
                dma_engines[b % 2].dma_start(
                    out=imf, in_=imc[:, 4 * b:4 * b + 4, :, :])
                imv = imf.rearrange("p s y (v c) -> p s y v c", c=2)

                H1 = H1s[b % 2]
                for s in range(4):
                    pt = big.tile([64, 1024], f32, tag="mm")
                    ptv = pt.rearrange("p (y x) -> p y x", x=32)
                    for m in range(2):
                        for oyh in range(2):
                            nc.tensor.matmul(
                                ptv[:, oyh * 16:(oyh + 1) * 16, :],
                                pk_at("c1w", m * 64, 64, rows=96),
                                imv[:, s, oyh * 16:(oyh + 1) * 16,
                                    m:m + 32, 0],
                                start=(m == 0), stop=(m == 1),
                            )
                    # j=0 half at x+1 (ACT/DVE alternating)
                    if s % 3 != 2:
                        nc.scalar.activation(out=H1[0:64, s, 1:33, 1:33],
                                             in_=ptv,
                                             func=AF.Prelu, alpha=0.2,
                                             scale=SC1EV)
                    else:
                        tmp = trash.tile([64, 1024], f16, tag="lr")
                        nc.vector.tensor_scalar(out=tmp, in0=ptv,
                                                scalar1=0.2 * SC1EV,
                                                scalar2=None, op0=OP.mult)
                        nc.vector.scalar_tensor_tensor(
                            out=H1[0:64, s, 1:33, 1:33],
                            in0=ptv, scalar=SC1EV,
                            in1=tmp.rearrange("p (y x) -> p y x", x=32),
                            op0=OP.mult, op1=OP.max)
                # j=1 (x-shifted) copy for conv2's K=128 packing: one flat
                # shift-by-one DMA (single contiguous descriptor/partition)
                H1f = H1.rearrange("p s y x -> p (s y x)")
                (nc.sync if b % 2 == 0 else nc.gpsimd).dma_start(
                    out=H1f[64:128, 0:4623], in_=H1f[0:64, 1:4624])

                # conv2 on this block (two 2-sample tiles)
                H1v = H1.rearrange("p s (r a) (v c) -> p s r a v c", a=2, c=2)
                for sb in range(2):
                    b2 = b * 2 + sb
                    pt2 = hold.tile([128, 512], f32, tag="hold")
                    k = 0
                    for dy in range(4):
                        for m in range(2):
                            nc.tensor.matmul(
                                pt2,
                                pk_at("c2w", (dy * 2 + m) * 128, 128),
                                H1v[:, sb * 2:sb * 2 + 2,
                                    dy // 2:dy // 2 + 16, dy % 2,
                                    m:m + 16, 0],
                                start=(k == 0), stop=(k == 7),
                            )
                            k += 1
                    nc.vector.bn_stats(out=s2ch[:, b2, :], in_=pt2)
                    if b2 % 2 == 0:
                        nc.scalar.activation(out=C2R[:, b2, :], in_=pt2,
                                             func=AF.Identity)
                    else:
                        nc.vector.tensor_copy(C2R[:, b2, :], pt2)

            # ---- BN2 + lrelu -> H2 ----
            mv2 = sm.tile([128, 2], f32, tag="mv2")
            nc.vector.bn_aggr(out=mv2, in_=s2ch.rearrange("p c s -> p (c s)"))
            a2, d2 = bn_finalize_mv(mv2, col(0), col(1), EPS2)
            C2Rv = C2R.rearrange("p nb (s y x) -> p (nb s) y x", s=2, x=16)
            for h in range(2):
                nc.scalar.activation(
                    out=H2[:, h * 16:(h + 1) * 16, 1:17, 1:17],
                    in_=C2Rv[:, h * 16:(h + 1) * 16],
                    func=AF.Prelu, bias=d2[:, 0:1], scale=a2[:, 0:1], alpha=0.2)

            # ---- conv3: PSUM-resident per mc; evict only stats + own 4 ----
            H2v = H2.rearrange("p s (r a) (v c) -> p s r a v c", a=2, c=2)
            s3ch = sm.tile([128, 2, 4, 6], f32, tag="s3ch")
            for mc in range(2):
                tiles = []
                for half in range(2):
                    pt3 = big.tile([128, 1024], f32, tag="mm")
                    tiles.append(pt3)
                    k = 0
                    for dy in range(4):
                        for dx in range(4):
                            for q in range(2):
                                s0 = half * 16 + q * 8
                                nc.tensor.matmul(
                                    pt3[:, q * 512:(q + 1) * 512],
                                    pk_at("c3w",
                                          ((dy * 4 + dx) * 2 + mc) * 128, 128),
                                    H2v[:, s0:s0 + 8, dy // 2:dy // 2 + 8,
                                        dy % 2, dx // 2:dx // 2 + 8, dx % 2],
                                    start=(k == 0), stop=(k == 15),
                                )
                            k += 1
                    for q in range(2):
                        nc.vector.bn_stats(
                            out=s3ch[:, mc, 2 * half + q, :],
                            in_=pt3[:, q * 512:(q + 1) * 512])
                mv3 = sm.tile([128, 2], f32, tag="mv3")
                nc.vector.bn_aggr(out=mv3,
                                  in_=s3ch[:, mc].rearrange("p c s -> p (c s)"))
                a3, d3 = bn_finalize_mv(mv3, col(2 + mc), col(4 + mc), EPS3)
                nc.scalar.activation(
                    out=ENC[:, mc, :, :],
                    in_=tiles[0][:, 0:BPC * 64].rearrange(
                        "p (s x) -> p s x", s=BPC),
                    func=AF.Prelu, bias=d3[:, 0:1], scale=a3[:, 0:1], alpha=0.2)

        def rn_part(col, relu_bias, AUXT):
            for s in range(BPC):
                nc.vector.tensor_copy(excl[0:AUX, :],
                                      AUXT[:, s:s + 1].to_broadcast((AUX, 64)))
                acp = hold.tile([128, 512], f32, tag="hold")
                nc.tensor.matmul(acp[0:64, 0:256], ENC[:, 0, s, :],
                                 pk_at("w1t", 0, 256), start=True, stop=False)
                nc.tensor.matmul(acp[0:64, 0:256], ENC[:, 1, s, :],
                                 pk_at("w1t", 256, 256), start=False, stop=False)
                nc.tensor.matmul(acp[0:64, 0:256], pk_at("aro", 0, 64, rows=2),
                                 pk_at("exA", 0, 256, rows=2),
                                 start=False, stop=True)
                nc.tensor.matmul(acp[64:128, 0:256], ENC[:, 0, s, :],
                                 pk_at("w1b", 0, 256), start=True, stop=False)
                nc.tensor.matmul(acp[64:128, 0:256], ENC[:, 1, s, :],
                                 pk_at("w1b", 256, 256), start=False, stop=False)
                nc.tensor.matmul(acp[64:128, 0:256], excl,
                                 pk_at("exC", 0, 256, rows=11),
                                 start=False, stop=True)
                if rn_fp8:
                    ac8 = sm.tile([128, 2, 256], f8, tag="ac8")
                    nc.scalar.copy(ac8[:, 0, :], acp[:, 0:256])
                    nc.vector.scalar_tensor_tensor(
                        out=ac8[:, 1, :], in0=acp[:, 0:256], scalar=1.0,
                        in1=ac8[:, 0, :], op0=OP.mult, op1=OP.subtract)
                else:
                    ac = sm.tile([128, 256], f16, tag="ac")
                    nc.scalar.copy(ac, acp[:, 0:256])

                b0, b1, b2 = bufs
                # L1: S.T = AC.T @ G, relu -> b0 (mc outer: stationary reuse)
                for mc in range(2):
                    for t2 in range(4):
                        pt = big.tile([128, 1024], f32, tag="mm")
                        for h in range(2):
                            tt = t2 * 2 + h
                            if rn_fp8:
                                nc.tensor.matmul(
                                    pt[:, h * 512:(h + 1) * 512],
                                    ac8[:, :, mc * 128:(mc + 1) * 128],
                                    gsel8_at(tt),
                                    start=True, stop=True, perf_mode=DR)
                            else:
                                nc.tensor.matmul(
                                    pt[:, h * 512:(h + 1) * 512],
                                    ac[:, mc * 128:(mc + 1) * 128],
                                    pk_at("gsel", tt * 512, 512),
                                    start=True, stop=True)
                        relu_bias((0, 1, 0, 1, 0, 1, 0, 1)[mc * 4 + t2],
                                  b0[:, mc, t2 * 1024:(t2 + 1) * 1024], pt,
                                  None)
                # L2..L4 dual-fp8 DoubleRow (or kc-outer fp16)
                for li, (wnm, bci, src, dst) in enumerate((
                        ("gw2", 6, b0, b1), ("gw3", 8, b1, b2),
                        ("gw4", 10, b2, None))):
                    if li == 2:
                        p4 = sm.tile([128, 2, 4], f32, tag="p4")
                    for mc in range(2):
                        for t2 in range(4):
                            pt = big.tile([128, 1024], f32, tag="mm")
                            for h in range(2):
                                tt = t2 * 2 + h
                                for kc in range(2):
                                    if rn_fp8:
                                        nc.tensor.matmul(
                                            pt[:, h * 512:(h + 1) * 512],
                                            pk8_dual(wnm, kc, mc),
                                            bcast2(src[:, kc,
                                                       tt * 512:(tt + 1) * 512]),
                                            start=(kc == 0), stop=(kc == 1),
                                            perf_mode=DR)
                                    else:
                                        nc.tensor.matmul(
                                            pt[:, h * 512:(h + 1) * 512],
                                            pk_at(wnm,
                                                  (kc * 2 + mc) * 128, 128),
                                            src[:, kc,
                                                tt * 512:(tt + 1) * 512],
                                            start=(kc == 0), stop=(kc == 1))
                            sc = INV_SG if rn_fp8 else None
                            if li < 2:
                                relu_bias((0, 1, 0, 1, 0, 0, 1, 0)[mc * 4 + t2],
                                          dst[:, mc,
                                              t2 * 1024:(t2 + 1) * 1024],
                                          pt, col(bci + mc), scale=sc)
                            else:
                                tb = trash.tile([128, 1024], f16, tag="tr4")
                                relu_bias((0, 1, 0, 1, 0, 1, 0, 1)[mc * 4 + t2],
                                          tb, pt, col(bci + mc),
                                          accum=p4[:, mc, t2:t2 + 1], scale=sc)
                nc.vector.tensor_reduce(out=POOL[:, s, :], in_=p4,
                                        axis=mybir.AxisListType.X, op=OP.add)

            # ---- f MLP ----
            POOLh = st.tile([128, BPC, 2], f16, tag="poolh")
            nc.vector.tensor_copy(POOLh, POOL)
            F1 = st.tile([128, 2, BPC], f16, tag="f1")
            F2 = st.tile([128, 2, BPC], f16, tag="f2")
            for (wnm, bci, src, dst) in (("fw1", 12, POOLh, F1),
                                         ("fw2", 14, F1, F2)):
                for mc in range(2):
                    pt = hold.tile([128, 512], f32, tag="hold")
                    if src is POOLh:
                        r0, r1 = src[:, :, 0], src[:, :, 1]
                    else:
                        r0, r1 = src[:, 0, :], src[:, 1, :]
                    nc.tensor.matmul(pt[:, 0:BPC], pk_at(wnm, mc * 128, 128),
                                     r0, start=True, stop=False)
                    nc.tensor.matmul(pt[:, 0:BPC],
                                     pk_at(wnm, (2 + mc) * 128, 128),
                                     r1, start=False, stop=True)
                    relu_bias(0, dst[:, mc, :], pt[:, 0:BPC], col(bci + mc))
            pt = hold.tile([128, 512], f32, tag="hold")
            nc.tensor.matmul(pt[0:1, 0:BPC], pk_at("fw3", 0, 1), F2[:, 0, :],
                             start=True, stop=False)
            nc.tensor.matmul(pt[0:1, 0:BPC], pk_at("fw3", 1, 1), F2[:, 1, :],
                             start=False, stop=True)
            osb = sm.tile([1, BPC], f32, tag="osb")
            nc.scalar.activation(out=osb, in_=pt[0:1, 0:BPC], func=AF.Identity,
                                 bias=fb3_s[0:1, 0:1])
            nc.sync.dma_start(out=out[:, :], in_=osb)

        def body():
            nc.sync.dma_start(out=PK, in_=pk16.ap())
            nc.sync.dma_start(out=P32, in_=pk32.ap())
            nc.gpsimd.dma_start(out=PK8, in_=pk8d.ap())
            nc.sync.dma_start(out=excl[10:11, :],
                              in_=pk_at("aro", 0, 64, rows=1))
            for H1 in H1s:
                nc.vector.memset(H1[:, :, 0:1, :], 0.0)
                nc.vector.memset(H1[:, :, 33:34, :], 0.0)
                nc.vector.memset(H1[:, :, :, 0:1], 0.0)
                nc.vector.memset(H1[:, :, :, 33:34], 0.0)
            nc.vector.memset(H2[:, :, 0:1, :], 0.0)
            nc.vector.memset(H2[:, :, 17:18, :], 0.0)
            nc.vector.memset(H2[:, :, :, 0:1], 0.0)
            nc.vector.memset(H2[:, :, :, 17:18], 0.0)

            # consts transpose -> CONST_T [128, 20]
            CONST_T = st.tile([128, 20], f32, tag="constT")
            ptc = hold.tile([128, 512], f32, tag="hold")
            nc.tensor.transpose(ptc[:, 0:20], csb, id_s[0:20, 0:20])
            nc.vector.tensor_copy(CONST_T, ptc[:, 0:20])
            AUXT = st.tile([AUX, BPC], f16, tag="auxT")
            pta = hold.tile([128, 512], f32, tag="hold")
            nc.tensor.transpose(pta[0:AUX, 0:BPC], asb, id_s[0:BPC, 0:BPC])
            nc.vector.tensor_copy(AUXT, pta[0:AUX, 0:BPC])

            def col(i):
                return CONST_T[:, i:i + 1]

            zcol = sm.tile([128, 1], f32, tag="zcol")
            nc.vector.memset(zcol, 0.0)

            def relu_bias(eng, dst, psum, bias_col, accum=None, scale=None):
                # eng: 0=ACT, 1=DVE. relu(psum*scale + bias); the DVE path
                # with scale drops bias (asserted zero host-side).
                if eng is True:
                    eng = 0
                elif eng is False:
                    eng = 1
                if eng == 0:
                    nc.scalar.activation(
                        out=dst, in_=psum, func=AF.Relu,
                        bias=bias_col if bias_col is not None else 0.0,
                        scale=scale if scale is not None else 1.0,
                        accum_out=accum)
                elif accum is not None:
                    if scale is not None:
                        nc.vector.scalar_tensor_tensor(
                            out=dst, in0=psum, scalar=scale,
                            in1=zcol[:, 0:1].to_broadcast(
                                (psum.partition_size(), psum.free_size())),
                            op0=OP.mult, op1=OP.max, accum_out=accum)
                    else:
                        nc.vector.scalar_tensor_tensor(
                            out=dst, in0=psum,
                            scalar=bias_col if bias_col is not None else 0.0,
                            in1=zcol[:, 0:1].to_broadcast(
                                (psum.partition_size(), psum.free_size())),
                            op0=OP.add, op1=OP.max, accum_out=accum)
                else:
                    if scale is not None:
                        nc.vector.tensor_scalar(
                            out=dst, in0=psum, scalar1=scale,
                            scalar2=0.0, op0=OP.mult, op1=OP.max)
                    else:
                        nc.vector.tensor_scalar(
                            out=dst, in0=psum,
                            scalar1=bias_col if bias_col is not None else 0.0,
                            scalar2=0.0, op0=OP.add, op1=OP.max)

            if run_conv:
                conv_part(col)
            else:
                nc.vector.memset(ENC, 0.02)
            if run_rn:
                rn_part(col, relu_bias, AUXT)
            else:
                osb = sm.tile([1, BPC], f32, tag="osb")
                nc.vector.tensor_copy(osb, ENC[0:1, 0, :, 0])
                nc.sync.dma_start(out=out[:, :], in_=osb)

        if loop_n > 0:
            with tc.For_i(0, loop_n, 1):
                body()
        else:
            body()

    nc.compile()
    return nc


def _host_im2col(img):
    """img [B, 12, 64, 64] -> imc [96, B, 32, 66] fp8 (scaled by S_IMG).
    imc[dy*12+ic, s, oy, 1+u] = img[s, ic, 2*oy+dy-1, u] (j=0 rows),
    rows 48:96 = rows 0:48 shifted one column left (j=1), zero-padded."""
    img8 = np.asarray(img, np.float16)
    imc = np.zeros((96, B, 32, 66), np.float16)
    oy = np.arange(32)
    for dy in range(4):
        h = 2 * oy + dy - 1
        ok = (h >= 0) & (h < 64)
        # [s, ic, oy_ok, u] -> [ic, s, oy_ok, u]
        imc[dy * 12:(dy + 1) * 12, :, ok, 1:65] = \
            img8[:, :, h[ok], :].transpose(1, 0, 2, 3)
    imc[48:96, :, :, 0:65] = imc[0:48, :, :, 1:66]
    return imc


def _in_maps(inputs):
    pk, pk8, pk32 = _host_pack(inputs)
    img = np.asarray(inputs['input_image'], dtype=np.float32)
    aux = np.asarray(inputs['inputg_aux_v'], dtype=np.float32)
    imc0 = _host_im2col(img)
    in_maps = []
    for c in range(NCORES):
        own = list(range(c * BPC, (c + 1) * BPC))
        rest = [i for i in range(B) if i not in own]
        perm = own + rest
        p32c = pk32.copy()
        p32c[0:BPC, 161:161 + AUX] = aux[own]
        in_maps.append({'imc': np.ascontiguousarray(imc0[:, perm]),
                        'pk16': pk, 'pk8': pk8, 'pk32': p32c})
    return in_maps


def kernel(**inputs):
    from concourse.bass_utils import run_bass_kernel_spmd

    key = ("nc3", NCORES)
    if key not in _CACHE:
        _CACHE[key] = _build(NCORES)
    nc = _CACHE[key]

    in_maps = _in_maps(inputs)
    res = run_bass_kernel_spmd(nc, in_maps, core_ids=list(range(NCORES)))
    kernel.last_result = res
    outs = [res.results[c]["out"].reshape(BPC, 1) for c in range(NCORES)]
    return np.concatenate(outs, axis=0).astype(np.float32)


# revision 33
# speedup vs baseline: 1.4237x; 1.2490x over previous
"""Trainium2 Bass kernel v3 for nn_DCGAN_C (DCGAN conv backbone + relation net).

Key changes vs v2 (327.9us -> target ~150us):
- RN layers use dual-plane fp8 DoubleRow matmuls: each fp16 weight W is split
  host-side into W_hi = fp8(W*S) and W_lo = fp8(W*S - W_hi) and packed as the
  two DoubleRow K-planes; the moving operand is broadcast (stride-0 plane), so
  out = (W_hi + W_lo) @ x at 0.5 cyc/row with ~0.1% effective weight error.
  The PSUM carries factor S; evictions apply 1/S (ACT scale= / DVE mult-max).
  The L1 A+C expansion uses the same trick with an on-chip hi/lo split of AC.
- conv1's second (j=1, x-shifted) H1 copy is made by one SBUF->SBUF DMA per
  block instead of per-sample engine evictions.
- BN statistics via DVE bn_stats/bn_aggr (single pass, no Square pass).
- conv3 output stays resident in PSUM per mc-half; only channel stats (DVE)
  and the core's own 4 samples (ENC) are ever evicted.
- No collectives: every core runs the full-batch conv backbone (exact BN
  batch stats), RN only on its own 4 samples. Pure SPMD.
- `loop_n` wraps the body in tc.For_i for steady-state benchmarking.
"""
import os
import numpy as np
from contextlib import ExitStack

NCORES = 8
B = 32           # full batch (replicated conv backbone)
BPC = 4          # samples per core for the relation network
NUM = 64
AUX = 10
EPS = 1e-5
S_G = 32.0       # RN weight pre-scale for the fp8 hi/lo split
INV_SG = 1.0 / S_G
S_IMG = 1.0      # conv runs in fp16: neutral scales
S_C1 = 1.0
S_C2 = 1.0
S_C3 = 1.0
S_H1 = 1.0
S_H2 = 1.0
SC1EV = S_H1 / (S_IMG * S_C1)       # conv1 eviction scale
EPS2 = EPS * (S_H1 * S_C2) ** 2     # eps in scaled-psum units
EPS3 = EPS * (S_H2 * S_C3) ** 2

_CACHE = {}

# fp16 const-pack layout: (name, rows, width)
_P16 = [
    ("c1w", 96, 2 * 128),        # [j*48+dy*12+ic][m*128 + half*64 + oc]
    ("c2w", 128, 8 * 128),       # [j*64+ic][(dy*2+m)*128 + oc]
    ("c3w", 128, 32 * 128),      # [ic][((dy*4+dx)*2+mc)*128 + oc']
    ("w1t", 128, 2 * 256),       # [p][kc*256 + c]
    ("w1b", 128, 2 * 256),
    ("exA", 2, 256),
    ("exC", 11, 256),
    ("aro", 2, 64),              # row0 arange, row1 ones
    ("fw1", 128, 4 * 128),
    ("fw2", 128, 4 * 128),
    ("fw3", 128, 2),
]
# fp8e4 pack: dual hi/lo (pl-major within each 2*M slice)
# c1w8: per m in 2: [j*48+dy*12+ic][m*128 + pl*64 + oc]        (M=64)
# c2w8: per (dy,m): [j*64+ic][(dy*2+m)*256 + pl*128 + oc]      (M=128)
# c3w8: per (dy,dx,mc): [ic][((dy*4+dx)*2+mc)*256 + pl*128+oc] (M=128)
# gw*:  per (kc,mc):  [(kc*2+mc)*256 + pl*128 + m]
_P8 = [("gw2", 1024), ("gw3", 1024), ("gw4", 1024), ("gsel8", NUM * NUM)]
_P8_OFF = {}
_off8 = 0
for _nm, _w in _P8:
    _P8_OFF[_nm] = _off8
    _off8 += _w
P8_W = _off8
_P16_OFF = {}
_off = 0
for _nm, _r, _w in _P16:
    _P16_OFF[_nm] = _off
    _off += _w
P16_W = _off
# fp32 pack [32, 204]: consts rows 0-19 cols 0-128; ident [0:32,128:160];
# fb3 [0,160]; aux [0:BPC, 161:171]
P32_W = 204


def _dual8(W, scale):
    """Split W*scale into fp8 hi + lo planes (returned as fp8 arrays)."""
    import ml_dtypes
    f8 = ml_dtypes.float8_e4m3
    hi = (W * scale).astype(f8)
    lo = (W * scale - hi.astype(np.float32)).astype(f8)
    return hi, lo


def _host_pack(inputs):
    f16 = np.float16
    w1 = inputs['conv1_w'].astype(np.float32)
    w2 = inputs['conv2_w'].astype(np.float32)
    w3 = inputs['conv3_w'].astype(np.float32)
    gW1 = inputs['gW1'].astype(np.float32)
    gb1 = inputs['gb1'].astype(np.float32)
    # the dual-fp8 DVE eviction path drops layer biases (they are zero for
    # this model); assert that stays true.
    for nm in ('gb2', 'gb3', 'gb4'):
        assert np.abs(np.asarray(inputs[nm])).max() == 0.0, nm

    pk = np.zeros((128, P16_W), f16)

    def put(nm, arr):
        o = _P16_OFF[nm]
        r, w = arr.shape
        pk[0:r, o:o + w] = arr.astype(f16)

    c1 = np.zeros((96, 256), np.float32)
    for j in range(2):
        for dy in range(4):
            for m in range(2):
                blk = w1[:, :, dy, 2 * m + j].T
                r = j * 48 + dy * 12
                c1[r:r + 12, m * 128:m * 128 + 64] = blk
                c1[r:r + 12, m * 128 + 64:m * 128 + 128] = blk
    put("c1w", c1)

    c2 = np.zeros((128, 8 * 128), np.float32)
    for dy in range(4):
        for m in range(2):
            o = (dy * 2 + m) * 128
            for j in range(2):
                c2[j * 64:(j + 1) * 64, o:o + 128] = w2[:, :, dy, 2 * m + j].T
    put("c2w", c2)

    c3 = np.zeros((128, 32 * 128), np.float32)
    for dy in range(4):
        for dx in range(4):
            for mc in range(2):
                o = ((dy * 4 + dx) * 2 + mc) * 128
                c3[:, o:o + 128] = w3[mc * 128:(mc + 1) * 128, :, dy, dx].T
    put("c3w", c3 * S_C3)

    w1t = np.zeros((128, 512), np.float32)
    w1b = np.zeros((128, 512), np.float32)
    for kc in range(2):
        w1t[:, kc * 256:(kc + 1) * 256] = gW1[kc * 128:(kc + 1) * 128]
        w1b[:, kc * 256:(kc + 1) * 256] = gW1[257 + kc * 128:257 + (kc + 1) * 128]
    put("w1t", w1t)
    put("w1b", w1b)
    put("exA", np.stack([gW1[256], gb1], 0))
    put("exC", np.concatenate([gW1[513:523], gW1[523:524]], 0))
    put("aro", np.stack([np.arange(64, dtype=np.float32),
                         np.ones(64, np.float32)], 0))

    def sq(nm, W):
        o = np.zeros((128, 512), np.float32)
        for kc in range(2):
            for mc in range(2):
                o[:, (kc * 2 + mc) * 128:(kc * 2 + mc + 1) * 128] = \
                    W[kc * 128:(kc + 1) * 128, mc * 128:(mc + 1) * 128]
        put(nm, o)

    sq("fw1", inputs['fW1']); sq("fw2", inputs['fW2'])
    f3 = np.zeros((128, 2), np.float32)
    for kc in range(2):
        f3[:, kc] = inputs['fW3'][kc * 128:(kc + 1) * 128, 0]
    put("fw3", f3)

    G = np.zeros((128, NUM * NUM), np.float32)
    ii, jj = np.meshgrid(np.arange(NUM), np.arange(NUM), indexing='ij')
    p = (ii * NUM + jj).ravel()
    G[jj.ravel(), p] = 1.0
    G[64 + ii.ravel(), p] = 1.0

    import ml_dtypes
    pk8 = np.zeros((128, P8_W), ml_dtypes.float8_e4m3)

    def put8_dual(nm, grp, W, scale):
        hi, lo = _dual8(W, scale)
        r, m = hi.shape
        c0 = _P8_OFF[nm] + grp * 2 * m
        pk8[0:r, c0:c0 + m] = hi
        pk8[0:r, c0 + m:c0 + 2 * m] = lo

    for nm in ("gw2", "gw3", "gw4"):
        W = np.asarray(inputs['gW' + nm[-1]], np.float32)
        for kc in range(2):
            for mc in range(2):
                put8_dual(nm, kc * 2 + mc,
                          W[kc * 128:(kc + 1) * 128, mc * 128:(mc + 1) * 128],
                          S_G)
    pk8[:, _P8_OFF["gsel8"]:_P8_OFF["gsel8"] + NUM * NUM] = \
        G.astype(ml_dtypes.float8_e4m3)

    pk32 = np.zeros((32, P32_W), np.float32)
    pk32[0, 0:128] = S_H2 * np.asarray(inputs['bn2_g'], np.float32)
    pk32[1, 0:128] = S_H2 * np.asarray(inputs['bn2_b'], np.float32)
    pk32[2, 0:128] = inputs['bn3_g'][:128]; pk32[3, 0:128] = inputs['bn3_g'][128:]
    pk32[4, 0:128] = inputs['bn3_b'][:128]; pk32[5, 0:128] = inputs['bn3_b'][128:]
    pk32[6, 0:128] = inputs['gb2'][:128]; pk32[7, 0:128] = inputs['gb2'][128:]
    pk32[8, 0:128] = inputs['gb3'][:128]; pk32[9, 0:128] = inputs['gb3'][128:]
    pk32[10, 0:128] = inputs['gb4'][:128]; pk32[11, 0:128] = inputs['gb4'][128:]
    pk32[12, 0:128] = inputs['fb1'][:128]; pk32[13, 0:128] = inputs['fb1'][128:]
    pk32[14, 0:128] = inputs['fb2'][:128]; pk32[15, 0:128] = inputs['fb2'][128:]
    pk32[0:32, 128:160] = np.eye(32, dtype=np.float32)
    pk32[0, 160] = float(np.asarray(inputs['fb3']).reshape(-1)[0])
    return pk, pk8, pk32


def _build(num_devices, loop_n=0, section="all", rn_fp8=True):
    import concourse.bacc as bacc
    import concourse.mybir as mybir
    import concourse.tile as tile

    f16 = mybir.dt.float16
    f32 = mybir.dt.float32
    f8 = mybir.dt.float8e4
    AF = mybir.ActivationFunctionType
    OP = mybir.AluOpType
    DR = mybir.MatmulPerfMode.DoubleRow

    NB = B // 4                  # conv1/conv2 4-sample blocks
    run_conv = section in ("all", "conv")
    run_rn = section in ("all", "rn")

    nc = bacc.Bacc(None, target_bir_lowering=False, num_devices=num_devices)

    # imc = host-side im2col of the (permuted) image batch:
    # imc[j*48+dy*12+ic, s, oy, 1-j+u] = img[s, ic, 2*oy+dy-1, u], zero-padded
    imc = nc.dram_tensor("imc", [96, B, 32, 66], f16, kind="ExternalInput")
    pk16 = nc.dram_tensor("pk16", [128, P16_W], f16, kind="ExternalInput")
    pk8d = nc.dram_tensor("pk8", [128, P8_W], f8, kind="ExternalInput")
    pk32 = nc.dram_tensor("pk32", [32, P32_W], f32, kind="ExternalInput")
    out = nc.dram_tensor("out", [1, BPC], f32, kind="ExternalOutput")

    with tile.TileContext(nc) as tc, ExitStack() as ctx:
        cw = ctx.enter_context(tc.tile_pool(name="cw", bufs=1))
        st = ctx.enter_context(tc.tile_pool(name="st", bufs=1))
        sm = ctx.enter_context(tc.tile_pool(name="sm", bufs=2))
        trash = ctx.enter_context(tc.tile_pool(name="trash", bufs=4))
        big = ctx.enter_context(tc.tile_pool(name="big", bufs=3, space="PSUM"))
        hold = ctx.enter_context(tc.tile_pool(name="hold", bufs=2, space="PSUM"))

        # ---- persistent tile allocations (all DMAs/memsets live in body) ----
        PK = cw.tile([128, P16_W], f16, name="PK")
        PK8 = cw.tile([128, P8_W], f8, name="PK8")
        P32 = cw.tile([32, P32_W], f32, name="P32")

        def pk_at(nm, o0, width, rows=128, r0=0):
            o = _P16_OFF[nm] + o0
            return PK[r0:r0 + rows, o:o + width]

        def pk8_dual(nm, kc, mc):
            o = _P8_OFF[nm] + (kc * 2 + mc) * 256
            return PK8[:, o:o + 256].rearrange("p (pl m) -> p pl m", pl=2)

        def gsel8_at(tt):
            o = _P8_OFF["gsel8"] + tt * 512
            return PK8[:, o:o + 512].rearrange(
                "p (o n) -> p o n", o=1).broadcast_to([128, 2, 512])

        def bcast2(ap):
            # [p, n] -> [p, 2, n] stride-0 plane for dual-fp8 DoubleRow
            return ap.rearrange("p (o n) -> p o n", o=1).broadcast_to(
                [ap.partition_size(), 2, ap.free_size()])

        csb = P32[0:20, 0:128]
        id_s = P32[0:32, 128:160]
        fb3_s = P32[0:1, 160:161]
        asb = P32[0:BPC, 161:161 + AUX]

        imfs = [st.tile([96, 4, 32, 66], f16, name=f"imf{t}") for t in range(2)]
        H1s = [st.tile([128, 4, 34, 34], f16, name=f"H1{t}") for t in range(2)]
        H2 = st.tile([128, B, 18, 18], f16, name="H2")
        C2R = st.tile([128, B // 2, 512], f16, name="C2R")  # raw conv2 (s,y,x)
        ENC = st.tile([128, 2, BPC, 64], f16, name="ENC")
        excl = st.tile([11, 64], f16, name="excl")
        rn_dt = f8 if rn_fp8 else f16
        bufs = [st.tile([128, 2, NUM * NUM], rn_dt, name=f"rnbuf{i}")
                for i in range(3)]
        POOL = st.tile([128, BPC, 2], f32, name="POOL")

        dma_engines = [nc.sync, nc.gpsimd]

        def pk8_conv(nm, grp, rows=128, m=128):
            o = _P8_OFF[nm] + grp * 2 * m
            return PK8[0:rows, o:o + 2 * m].rearrange(
                "p (pl m) -> p pl m", pl=2)

        def bn_finalize_mv(mv, g_col, b_col, eps_v):
            # mv = [128, 2] (mean, biased var) from bn_aggr
            std = sm.tile([128, 1], f32, tag="bns")
            epsb = sm.tile([128, 1], f32, tag="bne")
            nc.vector.memset(epsb, eps_v)
            nc.scalar.activation(out=std, in_=mv[:, 1:2], func=AF.Sqrt,
                                 bias=epsb[:, 0:1])
            rstd = sm.tile([128, 1], f32, tag="bnr")
            nc.vector.reciprocal(rstd, std)
            a_c = sm.tile([128, 1], f32, tag="bna")
            nc.vector.tensor_tensor(out=a_c, in0=rstd, in1=g_col, op=OP.mult)
            d_c = sm.tile([128, 1], f32, tag="bnd")
            nc.vector.tensor_tensor(out=d_c, in0=mv[:, 0:1], in1=a_c,
                                    op=OP.mult)
            nc.vector.tensor_tensor(out=d_c, in0=b_col, in1=d_c, op=OP.subtract)
            return a_c, d_c

        def conv_part(col):
            # ---- conv1 + conv2 pipeline over 4-sample blocks ----
            s2ch = sm.tile([128, 2 * NB, 6], f32, tag="s2ch")
            for b in range(NB):
                imf = imfs[b % 2]
                dma_engines[b % 2].dma_start(
                    out=imf, in_=imc[:, 4 * b:4 * b + 4, :, :])
                imv = imf.rearrange("p s y (v c) -> p s y v c", c=2)

                H1 = H1s[b % 2]
                for s in range(4):
                    pt = big.tile([128, 1024], f32, tag="mm")
                    ptv = pt.rearrange("p (y x) -> p y x", x=32)
                    for m in range(2):
                        for oyh in range(2):
                            nc.tensor.matmul(
                                ptv[:, oyh * 16:(oyh + 1) * 16, :],
                                pk_at("c1w", m * 128, 128, rows=96),
                                imv[:, s, oyh * 16:(oyh + 1) * 16,
                                    m:m + 32, 0],
                                start=(m == 0), stop=(m == 1),
                            )
                    # top half at x+1, bottom (j=1 copy for conv2) at x
                    nc.scalar.activation(out=H1[0:64, s, 1:33, 1:33],
                                         in_=ptv[0:64],
                                         func=AF.Prelu, alpha=0.2)
                    if s % 2 == 1:
                        tmp = trash.tile([64, 1024], f16, tag="lr")
                        nc.vector.tensor_scalar(out=tmp, in0=ptv[64:128],
                                                scalar1=0.2, scalar2=None,
                                                op0=OP.mult)
                        nc.vector.tensor_tensor(
                            out=H1[64:128, s, 1:33, 0:32],
                            in0=ptv[64:128],
                            in1=tmp.rearrange("p (y x) -> p y x", x=32),
                            op=OP.max)
                    else:
                        nc.scalar.activation(out=H1[64:128, s, 1:33, 0:32],
                                             in_=ptv[64:128],
                                             func=AF.Prelu, alpha=0.2)

                # conv2 on this block (two 2-sample tiles)
                H1v = H1.rearrange("p s (r a) (v c) -> p s r a v c", a=2, c=2)
                for sb in range(2):
                    b2 = b * 2 + sb
                    pt2 = hold.tile([128, 512], f32, tag="hold")
                    k = 0
                    for dy in range(4):
                        for m in range(2):
                            nc.tensor.matmul(
                                pt2,
                                pk_at("c2w", (dy * 2 + m) * 128, 128),
                                H1v[:, sb * 2:sb * 2 + 2,
                                    dy // 2:dy // 2 + 16, dy % 2,
                                    m:m + 16, 0],
                                start=(k == 0), stop=(k == 7),
                            )
                            k += 1
                    nc.vector.bn_stats(out=s2ch[:, b2, :], in_=pt2)
                    if b2 % 2 == 0:
                        nc.scalar.activation(out=C2R[:, b2, :], in_=pt2,
                                             func=AF.Identity)
                    else:
                        nc.vector.tensor_copy(C2R[:, b2, :], pt2)

            # ---- BN2 + lrelu -> H2 ----
            mv2 = sm.tile([128, 2], f32, tag="mv2")
            nc.vector.bn_aggr(out=mv2, in_=s2ch.rearrange("p c s -> p (c s)"))
            a2, d2 = bn_finalize_mv(mv2, col(0), col(1), EPS2)
            C2Rv = C2R.rearrange("p nb (s y x) -> p (nb s) y x", s=2, x=16)
            for h in range(2):
                nc.scalar.activation(
                    out=H2[:, h * 16:(h + 1) * 16, 1:17, 1:17],
                    in_=C2Rv[:, h * 16:(h + 1) * 16],
                    func=AF.Prelu, bias=d2[:, 0:1], scale=a2[:, 0:1], alpha=0.2)

            # ---- conv3: PSUM-resident per mc; evict only stats + own 4 ----
            H2v = H2.rearrange("p s (r a) (v c) -> p s r a v c", a=2, c=2)
            s3ch = sm.tile([128, 2, 4, 6], f32, tag="s3ch")
            for mc in range(2):
                tiles = []
                for half in range(2):
                    pt3 = big.tile([128, 1024], f32, tag="mm")
                    tiles.append(pt3)
                    k = 0
                    for dy in range(4):
                        for dx in range(4):
                            for q in range(2):
                                s0 = half * 16 + q * 8
                                nc.tensor.matmul(
                                    pt3[:, q * 512:(q + 1) * 512],
                                    pk_at("c3w",
                                          ((dy * 4 + dx) * 2 + mc) * 128, 128),
                                    H2v[:, s0:s0 + 8, dy // 2:dy // 2 + 8,
                                        dy % 2, dx // 2:dx // 2 + 8, dx % 2],
                                    start=(k == 0), stop=(k == 15),
                                )
                            k += 1
                    for q in range(2):
                        nc.vector.bn_stats(
                            out=s3ch[:, mc, 2 * half + q, :],
                            in_=pt3[:, q * 512:(q + 1) * 512])
                mv3 = sm.tile([128, 2], f32, tag="mv3")
                nc.vector.bn_aggr(out=mv3,
                                  in_=s3ch[:, mc].rearrange("p c s -> p (c s)"))
                a3, d3 = bn_finalize_mv(mv3, col(2 + mc), col(4 + mc), EPS3)
                nc.scalar.activation(
                    out=ENC[:, mc, :, :],
                    in_=tiles[0][:, 0:BPC * 64].rearrange(
                        "p (s x) -> p s x", s=BPC),
                    func=AF.Prelu, bias=d3[:, 0:1], scale=a3[:, 0:1], alpha=0.2)

        def rn_part(col, relu_bias, AUXT):
            for s in range(BPC):
                nc.vector.tensor_copy(excl[0:AUX, :],
                                      AUXT[:, s:s + 1].to_broadcast((AUX, 64)))
                acp = hold.tile([128, 512], f32, tag="hold")
                nc.tensor.matmul(acp[0:64, 0:256], ENC[:, 0, s, :],
                                 pk_at("w1t", 0, 256), start=True, stop=False)
                nc.tensor.matmul(acp[0:64, 0:256], ENC[:, 1, s, :],
                                 pk_at("w1t", 256, 256), start=False, stop=False)
                nc.tensor.matmul(acp[0:64, 0:256], pk_at("aro", 0, 64, rows=2),
                                 pk_at("exA", 0, 256, rows=2),
                                 start=False, stop=True)
                nc.tensor.matmul(acp[64:128, 0:256], ENC[:, 0, s, :],
                                 pk_at("w1b", 0, 256), start=True, stop=False)
                nc.tensor.matmul(acp[64:128, 0:256], ENC[:, 1, s, :],
                                 pk_at("w1b", 256, 256), start=False, stop=False)
                nc.tensor.matmul(acp[64:128, 0:256], excl,
                                 pk_at("exC", 0, 256, rows=11),
                                 start=False, stop=True)
                if rn_fp8:
                    ac8 = sm.tile([128, 2, 256], f8, tag="ac8")
                    nc.scalar.copy(ac8[:, 0, :], acp[:, 0:256])
                    nc.vector.scalar_tensor_tensor(
                        out=ac8[:, 1, :], in0=acp[:, 0:256], scalar=1.0,
                        in1=ac8[:, 0, :], op0=OP.mult, op1=OP.subtract)
                else:
                    ac = sm.tile([128, 256], f16, tag="ac")
                    nc.scalar.copy(ac, acp[:, 0:256])

                b0, b1, b2 = bufs
                # L1: S.T = AC.T @ G, relu -> b0 (mc outer: stationary reuse)
                for mc in range(2):
                    for t2 in range(4):
                        pt = big.tile([128, 1024], f32, tag="mm")
                        for h in range(2):
                            tt = t2 * 2 + h
                            if rn_fp8:
                                nc.tensor.matmul(
                                    pt[:, h * 512:(h + 1) * 512],
                                    ac8[:, :, mc * 128:(mc + 1) * 128],
                                    gsel8_at(tt),
                                    start=True, stop=True, perf_mode=DR)
                            else:
                                nc.tensor.matmul(
                                    pt[:, h * 512:(h + 1) * 512],
                                    ac[:, mc * 128:(mc + 1) * 128],
                                    pk_at("gsel", tt * 512, 512),
                                    start=True, stop=True)
                        relu_bias((0, 1, 0, 1, 0, 1, 0, 1)[mc * 4 + t2],
                                  b0[:, mc, t2 * 1024:(t2 + 1) * 1024], pt,
                                  None)
                # L2..L4 dual-fp8 DoubleRow (or kc-outer fp16)
                for li, (wnm, bci, src, dst) in enumerate((
                        ("gw2", 6, b0, b1), ("gw3", 8, b1, b2),
                        ("gw4", 10, b2, None))):
                    if li == 2:
                        p4 = sm.tile([128, 2, 4], f32, tag="p4")
                    for mc in range(2):
                        for t2 in range(4):
                            pt = big.tile([128, 1024], f32, tag="mm")
                            for h in range(2):
                                tt = t2 * 2 + h
                                for kc in range(2):
                                    if rn_fp8:
                                        nc.tensor.matmul(
                                            pt[:, h * 512:(h + 1) * 512],
                                            pk8_dual(wnm, kc, mc),
                                            bcast2(src[:, kc,
                                                       tt * 512:(tt + 1) * 512]),
                                            start=(kc == 0), stop=(kc == 1),
                                            perf_mode=DR)
                                    else:
                                        nc.tensor.matmul(
                                            pt[:, h * 512:(h + 1) * 512],
                                            pk_at(wnm,
                                                  (kc * 2 + mc) * 128, 128),
                                            src[:, kc,
                                                tt * 512:(tt + 1) * 512],
                                            start=(kc == 0), stop=(kc == 1))
                            sc = INV_SG if rn_fp8 else None
                            if li < 2:
                                relu_bias((0, 1, 0, 1, 0, 0, 1, 0)[mc * 4 + t2],
                                          dst[:, mc,
                                              t2 * 1024:(t2 + 1) * 1024],
                                          pt, col(bci + mc), scale=sc)
                            else:
                                tb = trash.tile([128, 1024], f16, tag="tr4")
                                relu_bias((0, 1, 0, 1, 0, 1, 0, 1)[mc * 4 + t2],
                                          tb, pt, col(bci + mc),
                                          accum=p4[:, mc, t2:t2 + 1], scale=sc)
                nc.vector.tensor_reduce(out=POOL[:, s, :], in_=p4,
                                        axis=mybir.AxisListType.X, op=OP.add)

            # ---- f MLP ----
            POOLh = st.tile([128, BPC, 2], f16, tag="poolh")
            nc.vector.tensor_copy(POOLh, POOL)
            F1 = st.tile([128, 2, BPC], f16, tag="f1")
            F2 = st.tile([128, 2, BPC], f16, tag="f2")
            for (wnm, bci, src, dst) in (("fw1", 12, POOLh, F1),
                                         ("fw2", 14, F1, F2)):
                for mc in range(2):
                    pt = hold.tile([128, 512], f32, tag="hold")
                    if src is POOLh:
                        r0, r1 = src[:, :, 0], src[:, :, 1]
                    else:
                        r0, r1 = src[:, 0, :], src[:, 1, :]
                    nc.tensor.matmul(pt[:, 0:BPC], pk_at(wnm, mc * 128, 128),
                                     r0, start=True, stop=False)
                    nc.tensor.matmul(pt[:, 0:BPC],
                                     pk_at(wnm, (2 + mc) * 128, 128),
                                     r1, start=False, stop=True)
                    relu_bias(0, dst[:, mc, :], pt[:, 0:BPC], col(bci + mc))
            pt = hold.tile([128, 512], f32, tag="hold")
            nc.tensor.matmul(pt[0:1, 0:BPC], pk_at("fw3", 0, 1), F2[:, 0, :],
                             start=True, stop=False)
            nc.tensor.matmul(pt[0:1, 0:BPC], pk_at("fw3", 1, 1), F2[:, 1, :],
                             start=False, stop=True)
            osb = sm.tile([1, BPC], f32, tag="osb")
            nc.scalar.activation(out=osb, in_=pt[0:1, 0:BPC], func=AF.Identity,
                                 bias=fb3_s[0:1, 0:1])
            nc.sync.dma_start(out=out[:, :], in_=osb)

        def body():
            nc.sync.dma_start(out=PK, in_=pk16.ap())
            nc.sync.dma_start(out=P32, in_=pk32.ap())
            nc.gpsimd.dma_start(out=PK8, in_=pk8d.ap())
            nc.sync.dma_start(out=excl[10:11, :],
                              in_=pk_at("aro", 0, 64, rows=1))
            for H1 in H1s:
                nc.vector.memset(H1[:, :, 0:1, :], 0.0)
                nc.vector.memset(H1[:, :, 33:34, :], 0.0)
                nc.vector.memset(H1[:, :, :, 0:1], 0.0)
                nc.vector.memset(H1[:, :, :, 33:34], 0.0)
                nc.vector.memset(H1[64:128, :, :, 32:33], 0.0)
            nc.vector.memset(H2[:, :, 0:1, :], 0.0)
            nc.vector.memset(H2[:, :, 17:18, :], 0.0)
            nc.vector.memset(H2[:, :, :, 0:1], 0.0)
            nc.vector.memset(H2[:, :, :, 17:18], 0.0)

            # consts transpose -> CONST_T [128, 20]
            CONST_T = st.tile([128, 20], f32, tag="constT")
            ptc = hold.tile([128, 512], f32, tag="hold")
            nc.tensor.transpose(ptc[:, 0:20], csb, id_s[0:20, 0:20])
            nc.vector.tensor_copy(CONST_T, ptc[:, 0:20])
            AUXT = st.tile([AUX, BPC], f16, tag="auxT")
            pta = hold.tile([128, 512], f32, tag="hold")
            nc.tensor.transpose(pta[0:AUX, 0:BPC], asb, id_s[0:BPC, 0:BPC])
            nc.vector.tensor_copy(AUXT, pta[0:AUX, 0:BPC])

            def col(i):
                return CONST_T[:, i:i + 1]

            zcol = sm.tile([128, 1], f32, tag="zcol")
            nc.vector.memset(zcol, 0.0)

            def relu_bias(eng, dst, psum, bias_col, accum=None, scale=None):
                # eng: 0=ACT, 1=DVE. relu(psum*scale + bias); the DVE path
                # with scale drops bias (asserted zero host-side).
                if eng is True:
                    eng = 0
                elif eng is False:
                    eng = 1
                if eng == 0:
                    nc.scalar.activation(
                        out=dst, in_=psum, func=AF.Relu,
                        bias=bias_col if bias_col is not None else 0.0,
                        scale=scale if scale is not None else 1.0,
                        accum_out=accum)
                elif accum is not None:
                    if scale is not None:
                        nc.vector.scalar_tensor_tensor(
                            out=dst, in0=psum, scalar=scale,
                            in1=zcol[:, 0:1].to_broadcast(
                                (psum.partition_size(), psum.free_size())),
                            op0=OP.mult, op1=OP.max, accum_out=accum)
                    else:
                        nc.vector.scalar_tensor_tensor(
                            out=dst, in0=psum,
                            scalar=bias_col if bias_col is not None else 0.0,
                            in1=zcol[:, 0:1].to_broadcast(
                                (psum.partition_size(), psum.free_size())),
                            op0=OP.add, op1=OP.max, accum_out=accum)
                else:
                    if scale is not None:
                        nc.vector.tensor_scalar(
                            out=dst, in0=psum, scalar1=scale,
                            scalar2=0.0, op0=OP.mult, op1=OP.max)
                    else:
                        nc.vector.tensor_scalar(
                            out=dst, in0=psum,
                            scalar1=bias_col if bias_col is not None else 0.0,
                            scalar2=0.0, op0=OP.add, op1=OP.max)

            if run_conv:
                conv_part(col)
            else:
                nc.vector.memset(ENC, 0.02)
            if run_rn:
                rn_part(col, relu_bias, AUXT)
            else:
                osb = sm.tile([1, BPC], f32, tag="osb")
                nc.vector.tensor_copy(osb, ENC[0:1, 0, :, 0])
                nc.sync.dma_start(out=out[:, :], in_=osb)

        if loop_n > 0:
            with tc.For_i(0, loop_n, 1):
                body()
        else:
            body()

    nc.compile()
    return nc


def _host_im2col(img):
    """img [B, 12, 64, 64] -> imc [96, B, 32, 66] fp8 (scaled by S_IMG).
    imc[dy*12+ic, s, oy, 1+u] = img[s, ic, 2*oy+dy-1, u] (j=0 rows),
    rows 48:96 = rows 0:48 shifted one column left (j=1), zero-padded."""
    img8 = np.asarray(img, np.float16)
    imc = np.zeros((96, B, 32, 66), np.float16)
    oy = np.arange(32)
    for dy in range(4):
        h = 2 * oy + dy - 1
        ok = (h >= 0) & (h < 64)
        # [s, ic, oy_ok, u] -> [ic, s, oy_ok, u]
        imc[dy * 12:(dy + 1) * 12, :, ok, 1:65] = \
            img8[:, :, h[ok], :].transpose(1, 0, 2, 3)
    imc[48:96, :, :, 0:65] = imc[0:48, :, :, 1:66]
    return imc


def _in_maps(inputs):
    pk, pk8, pk32 = _host_pack(inputs)
    img = np.asarray(inputs['input_image'], dtype=np.float32)
    aux = np.asarray(inputs['inputg_aux_v'], dtype=np.float32)
    imc0 = _host_im2col(img)
    in_maps = []
    for c in range(NCORES):
        own = list(range(c * BPC, (c + 1) * BPC))
        rest = [i for i in range(B) if i not in own]
        perm = own + rest
        p32c = pk32.copy()
        p32c[0:BPC, 161:161 + AUX] = aux[own]
        in_maps.append({'imc': np.ascontiguousarray(imc0[:, perm]),
                        'pk16': pk, 'pk8': pk8, 'pk32': p32c})
    return in_maps


def kernel(**inputs):
    from concourse.bass_utils import run_bass_kernel_spmd

    key = ("nc3", NCORES)
    if key not in _CACHE:
        _CACHE[key] = _build(NCORES)
    nc = _CACHE[key]

    in_maps = _in_maps(inputs)
    res = run_bass_kernel_spmd(nc, in_maps, core_ids=list(range(NCORES)))
    kernel.last_result = res
    outs = [res.results[c]["out"].reshape(BPC, 1) for c in range(NCORES)]
    return np.concatenate(outs, axis=0).astype(np.float32)


# revision 34
# speedup vs baseline: 1.4659x; 1.0296x over previous
"""Trainium2 Bass kernel v3 for nn_DCGAN_C (DCGAN conv backbone + relation net).

Key changes vs v2 (327.9us -> target ~150us):
- RN layers use dual-plane fp8 DoubleRow matmuls: each fp16 weight W is split
  host-side into W_hi = fp8(W*S) and W_lo = fp8(W*S - W_hi) and packed as the
  two DoubleRow K-planes; the moving operand is broadcast (stride-0 plane), so
  out = (W_hi + W_lo) @ x at 0.5 cyc/row with ~0.1% effective weight error.
  The PSUM carries factor S; evictions apply 1/S (ACT scale= / DVE mult-max).
  The L1 A+C expansion uses the same trick with an on-chip hi/lo split of AC.
- conv1's second (j=1, x-shifted) H1 copy is made by one SBUF->SBUF DMA per
  block instead of per-sample engine evictions.
- BN statistics via DVE bn_stats/bn_aggr (single pass, no Square pass).
- conv3 output stays resident in PSUM per mc-half; only channel stats (DVE)
  and the core's own 4 samples (ENC) are ever evicted.
- No collectives: every core runs the full-batch conv backbone (exact BN
  batch stats), RN only on its own 4 samples. Pure SPMD.
- `loop_n` wraps the body in tc.For_i for steady-state benchmarking.
"""
import os
import numpy as np
from contextlib import ExitStack

NCORES = 8
B = 32           # full batch (replicated conv backbone)
BPC = 4          # samples per core for the relation network
NUM = 64
AUX = 10
EPS = 1e-5
S_G = 8.0        # RN weight pre-scale for the fp8 hi/lo split
INV_SG = 1.0 / S_G
S_IMG = 1.0      # conv runs in fp16: neutral scales
S_C1 = 1.0
S_C2 = 1.0
S_C3 = 1.0
S_H1 = 1.0
S_H2 = 1.0
SC1EV = S_H1 / (S_IMG * S_C1)       # conv1 eviction scale
EPS2 = EPS * (S_H1 * S_C2) ** 2     # eps in scaled-psum units
EPS3 = EPS * (S_H2 * S_C3) ** 2

_CACHE = {}

# fp16 const-pack layout: (name, rows, width)
_P16 = [
    ("c1w", 96, 2 * 128),        # [j*48+dy*12+ic][m*128 + half*64 + oc]
    ("c2w", 128, 8 * 128),       # [j*64+ic][(dy*2+m)*128 + oc]
    ("c3w", 128, 32 * 128),      # [ic][((dy*4+dx)*2+mc)*128 + oc']
    ("w1t", 128, 2 * 256),       # [p][kc*256 + c]
    ("w1b", 128, 2 * 256),
    ("exA", 2, 256),
    ("exC", 11, 256),
    ("aro", 2, 64),              # row0 arange, row1 ones
    ("fw1", 128, 4 * 128),
    ("fw2", 128, 4 * 128),
    ("fw3", 128, 2),
]
# fp8e4 pack: dual hi/lo (pl-major within each 2*M slice)
# c1w8: per m in 2: [j*48+dy*12+ic][m*128 + pl*64 + oc]        (M=64)
# c2w8: per (dy,m): [j*64+ic][(dy*2+m)*256 + pl*128 + oc]      (M=128)
# c3w8: per (dy,dx,mc): [ic][((dy*4+dx)*2+mc)*256 + pl*128+oc] (M=128)
# gw*:  per (kc,mc):  [(kc*2+mc)*256 + pl*128 + m]
_P8 = [("gw2", 1024), ("gw3", 1024), ("gw4", 1024), ("gsel8", NUM * NUM)]
_P8_OFF = {}
_off8 = 0
for _nm, _w in _P8:
    _P8_OFF[_nm] = _off8
    _off8 += _w
P8_W = _off8
_P16_OFF = {}
_off = 0
for _nm, _r, _w in _P16:
    _P16_OFF[_nm] = _off
    _off += _w
P16_W = _off
# fp32 pack [32, 204]: consts rows 0-19 cols 0-128; ident [0:32,128:160];
# fb3 [0,160]; aux [0:BPC, 161:171]
P32_W = 204


def _dual8(W, scale):
    """Split W*scale into fp8 hi + lo planes (returned as fp8 arrays)."""
    import ml_dtypes
    f8 = ml_dtypes.float8_e4m3
    hi = (W * scale).astype(f8)
    lo = (W * scale - hi.astype(np.float32)).astype(f8)
    return hi, lo


def _host_pack(inputs):
    f16 = np.float16
    w1 = inputs['conv1_w'].astype(np.float32)
    w2 = inputs['conv2_w'].astype(np.float32)
    w3 = inputs['conv3_w'].astype(np.float32)
    gW1 = inputs['gW1'].astype(np.float32)
    gb1 = inputs['gb1'].astype(np.float32)
    # the dual-fp8 DVE eviction path drops layer biases (they are zero for
    # this model); assert that stays true.
    for nm in ('gb2', 'gb3', 'gb4'):
        assert np.abs(np.asarray(inputs[nm])).max() == 0.0, nm

    pk = np.zeros((128, P16_W), f16)

    def put(nm, arr):
        o = _P16_OFF[nm]
        r, w = arr.shape
        pk[0:r, o:o + w] = arr.astype(f16)

    c1 = np.zeros((96, 256), np.float32)
    for j in range(2):
        for dy in range(4):
            for m in range(2):
                blk = w1[:, :, dy, 2 * m + j].T
                r = j * 48 + dy * 12
                c1[r:r + 12, m * 128:m * 128 + 64] = blk
                c1[r:r + 12, m * 128 + 64:m * 128 + 128] = blk
    put("c1w", c1)

    c2 = np.zeros((128, 8 * 128), np.float32)
    for dy in range(4):
        for m in range(2):
            o = (dy * 2 + m) * 128
            for j in range(2):
                c2[j * 64:(j + 1) * 64, o:o + 128] = w2[:, :, dy, 2 * m + j].T
    put("c2w", c2)

    c3 = np.zeros((128, 32 * 128), np.float32)
    for dy in range(4):
        for dx in range(4):
            for mc in range(2):
                o = ((dy * 4 + dx) * 2 + mc) * 128
                c3[:, o:o + 128] = w3[mc * 128:(mc + 1) * 128, :, dy, dx].T
    put("c3w", c3 * S_C3)

    w1t = np.zeros((128, 512), np.float32)
    w1b = np.zeros((128, 512), np.float32)
    for kc in range(2):
        w1t[:, kc * 256:(kc + 1) * 256] = gW1[kc * 128:(kc + 1) * 128]
        w1b[:, kc * 256:(kc + 1) * 256] = gW1[257 + kc * 128:257 + (kc + 1) * 128]
    put("w1t", w1t)
    put("w1b", w1b)
    put("exA", np.stack([gW1[256], gb1], 0))
    put("exC", np.concatenate([gW1[513:523], gW1[523:524]], 0))
    put("aro", np.stack([np.arange(64, dtype=np.float32),
                         np.ones(64, np.float32)], 0))

    def sq(nm, W):
        o = np.zeros((128, 512), np.float32)
        for kc in range(2):
            for mc in range(2):
                o[:, (kc * 2 + mc) * 128:(kc * 2 + mc + 1) * 128] = \
                    W[kc * 128:(kc + 1) * 128, mc * 128:(mc + 1) * 128]
        put(nm, o)

    sq("fw1", inputs['fW1']); sq("fw2", inputs['fW2'])
    f3 = np.zeros((128, 2), np.float32)
    for kc in range(2):
        f3[:, kc] = inputs['fW3'][kc * 128:(kc + 1) * 128, 0]
    put("fw3", f3)

    G = np.zeros((128, NUM * NUM), np.float32)
    ii, jj = np.meshgrid(np.arange(NUM), np.arange(NUM), indexing='ij')
    p = (ii * NUM + jj).ravel()
    G[jj.ravel(), p] = 1.0
    G[64 + ii.ravel(), p] = 1.0

    import ml_dtypes
    pk8 = np.zeros((128, P8_W), ml_dtypes.float8_e4m3)

    def put8_dual(nm, grp, W, scale):
        hi, lo = _dual8(W, scale)
        r, m = hi.shape
        c0 = _P8_OFF[nm] + grp * 2 * m
        pk8[0:r, c0:c0 + m] = hi
        pk8[0:r, c0 + m:c0 + 2 * m] = lo

    for nm in ("gw2", "gw3", "gw4"):
        W = np.asarray(inputs['gW' + nm[-1]], np.float32)
        for kc in range(2):
            for mc in range(2):
                put8_dual(nm, kc * 2 + mc,
                          W[kc * 128:(kc + 1) * 128, mc * 128:(mc + 1) * 128],
                          S_G)
    pk8[:, _P8_OFF["gsel8"]:_P8_OFF["gsel8"] + NUM * NUM] = \
        G.astype(ml_dtypes.float8_e4m3)

    pk32 = np.zeros((32, P32_W), np.float32)
    pk32[0, 0:128] = S_H2 * np.asarray(inputs['bn2_g'], np.float32)
    pk32[1, 0:128] = S_H2 * np.asarray(inputs['bn2_b'], np.float32)
    pk32[2, 0:128] = inputs['bn3_g'][:128]; pk32[3, 0:128] = inputs['bn3_g'][128:]
    pk32[4, 0:128] = inputs['bn3_b'][:128]; pk32[5, 0:128] = inputs['bn3_b'][128:]
    pk32[6, 0:128] = inputs['gb2'][:128]; pk32[7, 0:128] = inputs['gb2'][128:]
    pk32[8, 0:128] = inputs['gb3'][:128]; pk32[9, 0:128] = inputs['gb3'][128:]
    pk32[10, 0:128] = inputs['gb4'][:128]; pk32[11, 0:128] = inputs['gb4'][128:]
    pk32[12, 0:128] = inputs['fb1'][:128]; pk32[13, 0:128] = inputs['fb1'][128:]
    pk32[14, 0:128] = inputs['fb2'][:128]; pk32[15, 0:128] = inputs['fb2'][128:]
    pk32[0:32, 128:160] = np.eye(32, dtype=np.float32)
    pk32[0, 160] = float(np.asarray(inputs['fb3']).reshape(-1)[0])
    return pk, pk8, pk32


def _build(num_devices, loop_n=0, section="all", rn_fp8=True):
    import concourse.bacc as bacc
    import concourse.mybir as mybir
    import concourse.tile as tile

    f16 = mybir.dt.float16
    f32 = mybir.dt.float32
    f8 = mybir.dt.float8e4
    AF = mybir.ActivationFunctionType
    OP = mybir.AluOpType
    DR = mybir.MatmulPerfMode.DoubleRow

    NB = B // 4                  # conv1/conv2 4-sample blocks
    run_conv = section in ("all", "conv")
    run_rn = section in ("all", "rn")

    nc = bacc.Bacc(None, target_bir_lowering=False, num_devices=num_devices)

    # imc = host-side im2col of the (permuted) image batch:
    # imc[j*48+dy*12+ic, s, oy, 1-j+u] = img[s, ic, 2*oy+dy-1, u], zero-padded
    imc = nc.dram_tensor("imc", [96, B, 32, 66], f16, kind="ExternalInput")
    pk16 = nc.dram_tensor("pk16", [128, P16_W], f16, kind="ExternalInput")
    pk8d = nc.dram_tensor("pk8", [128, P8_W], f8, kind="ExternalInput")
    pk32 = nc.dram_tensor("pk32", [32, P32_W], f32, kind="ExternalInput")
    out = nc.dram_tensor("out", [1, BPC], f32, kind="ExternalOutput")

    with tile.TileContext(nc) as tc, ExitStack() as ctx:
        cw = ctx.enter_context(tc.tile_pool(name="cw", bufs=1))
        st = ctx.enter_context(tc.tile_pool(name="st", bufs=1))
        sm = ctx.enter_context(tc.tile_pool(name="sm", bufs=2))
        trash = ctx.enter_context(tc.tile_pool(name="trash", bufs=4))
        big = ctx.enter_context(tc.tile_pool(name="big", bufs=3, space="PSUM"))
        hold = ctx.enter_context(tc.tile_pool(name="hold", bufs=2, space="PSUM"))

        # ---- persistent tile allocations (all DMAs/memsets live in body) ----
        PK = cw.tile([128, P16_W], f16, name="PK")
        PK8 = cw.tile([128, P8_W], f8, name="PK8")
        P32 = cw.tile([32, P32_W], f32, name="P32")

        def pk_at(nm, o0, width, rows=128, r0=0):
            o = _P16_OFF[nm] + o0
            return PK[r0:r0 + rows, o:o + width]

        def pk8_dual(nm, kc, mc):
            o = _P8_OFF[nm] + (kc * 2 + mc) * 256
            return PK8[:, o:o + 256].rearrange("p (pl m) -> p pl m", pl=2)

        def gsel8_at(tt):
            o = _P8_OFF["gsel8"] + tt * 512
            return PK8[:, o:o + 512].rearrange(
                "p (o n) -> p o n", o=1).broadcast_to([128, 2, 512])

        def bcast2(ap):
            # [p, n] -> [p, 2, n] stride-0 plane for dual-fp8 DoubleRow
            return ap.rearrange("p (o n) -> p o n", o=1).broadcast_to(
                [ap.partition_size(), 2, ap.free_size()])

        csb = P32[0:20, 0:128]
        id_s = P32[0:32, 128:160]
        fb3_s = P32[0:1, 160:161]
        asb = P32[0:BPC, 161:161 + AUX]

        imfs = [st.tile([96, 4, 32, 66], f16, name=f"imf{t}") for t in range(2)]
        H1s = [st.tile([128, 4, 34, 34], f16, name=f"H1{t}") for t in range(2)]
        H2 = st.tile([128, B, 18, 18], f16, name="H2")
        C2R = st.tile([128, B // 2, 512], f16, name="C2R")  # raw conv2 (s,y,x)
        ENC = st.tile([128, 2, BPC, 64], f16, name="ENC")
        excl = st.tile([11, 64], f16, name="excl")
        rn_dt = f8 if rn_fp8 else f16
        bufs = [st.tile([128, 2, NUM * NUM], rn_dt, name=f"rnbuf{i}")
                for i in range(3)]
        POOL = st.tile([128, BPC, 2], f32, name="POOL")

        dma_engines = [nc.sync, nc.gpsimd]

        def pk8_conv(nm, grp, rows=128, m=128):
            o = _P8_OFF[nm] + grp * 2 * m
            return PK8[0:rows, o:o + 2 * m].rearrange(
                "p (pl m) -> p pl m", pl=2)

        def bn_finalize_mv(mv, g_col, b_col, eps_v):
            # mv = [128, 2] (mean, biased var) from bn_aggr
            std = sm.tile([128, 1], f32, tag="bns")
            epsb = sm.tile([128, 1], f32, tag="bne")
            nc.vector.memset(epsb, eps_v)
            nc.scalar.activation(out=std, in_=mv[:, 1:2], func=AF.Sqrt,
                                 bias=epsb[:, 0:1])
            rstd = sm.tile([128, 1], f32, tag="bnr")
            nc.vector.reciprocal(rstd, std)
            a_c = sm.tile([128, 1], f32, tag="bna")
            nc.vector.tensor_tensor(out=a_c, in0=rstd, in1=g_col, op=OP.mult)
            d_c = sm.tile([128, 1], f32, tag="bnd")
            nc.vector.tensor_tensor(out=d_c, in0=mv[:, 0:1], in1=a_c,
                                    op=OP.mult)
            nc.vector.tensor_tensor(out=d_c, in0=b_col, in1=d_c, op=OP.subtract)
            return a_c, d_c

        def conv_part(col):
            # ---- conv1 + conv2 pipeline over 4-sample blocks ----
            s2ch = sm.tile([128, 2 * NB, 6], f32, tag="s2ch")
            for b in range(NB):
                imf = imfs[b % 2]
                dma_engines[b % 2].dma_start(
                    out=imf, in_=imc[:, 4 * b:4 * b + 4, :, :])
                imv = imf.rearrange("p s y (v c) -> p s y v c", c=2)

                H1 = H1s[b % 2]
                for s in range(4):
                    pt = big.tile([128, 1024], f32, tag="mm")
                    ptv = pt.rearrange("p (y x) -> p y x", x=32)
                    for m in range(2):
                        for oyh in range(2):
                            nc.tensor.matmul(
                                ptv[:, oyh * 16:(oyh + 1) * 16, :],
                                pk_at("c1w", m * 128, 128, rows=96),
                                imv[:, s, oyh * 16:(oyh + 1) * 16,
                                    m:m + 32, 0],
                                start=(m == 0), stop=(m == 1),
                            )
                    # top half at x+1, bottom (j=1 copy for conv2) at x
                    nc.scalar.activation(out=H1[0:64, s, 1:33, 1:33],
                                         in_=ptv[0:64],
                                         func=AF.Prelu, alpha=0.2)
                    if s % 2 == 1:
                        tmp = trash.tile([64, 1024], f16, tag="lr")
                        nc.vector.tensor_scalar(out=tmp, in0=ptv[64:128],
                                                scalar1=0.2, scalar2=None,
                                                op0=OP.mult)
                        nc.vector.tensor_tensor(
                            out=H1[64:128, s, 1:33, 0:32],
                            in0=ptv[64:128],
                            in1=tmp.rearrange("p (y x) -> p y x", x=32),
                            op=OP.max)
                    else:
                        nc.scalar.activation(out=H1[64:128, s, 1:33, 0:32],
                                             in_=ptv[64:128],
                                             func=AF.Prelu, alpha=0.2)

                # conv2 on this block (two 2-sample tiles)
                H1v = H1.rearrange("p s (r a) (v c) -> p s r a v c", a=2, c=2)
                for sb in range(2):
                    b2 = b * 2 + sb
                    pt2 = hold.tile([128, 512], f32, tag="hold")
                    k = 0
                    for dy in range(4):
                        for m in range(2):
                            nc.tensor.matmul(
                                pt2,
                                pk_at("c2w", (dy * 2 + m) * 128, 128),
                                H1v[:, sb * 2:sb * 2 + 2,
                                    dy // 2:dy // 2 + 16, dy % 2,
                                    m:m + 16, 0],
                                start=(k == 0), stop=(k == 7),
                            )
                            k += 1
                    nc.vector.bn_stats(out=s2ch[:, b2, :], in_=pt2)
                    if b2 % 2 == 0:
                        nc.scalar.activation(out=C2R[:, b2, :], in_=pt2,
                                             func=AF.Identity)
                    else:
                        nc.vector.tensor_copy(C2R[:, b2, :], pt2)

            # ---- BN2 + lrelu -> H2 ----
            mv2 = sm.tile([128, 2], f32, tag="mv2")
            nc.vector.bn_aggr(out=mv2, in_=s2ch.rearrange("p c s -> p (c s)"))
            a2, d2 = bn_finalize_mv(mv2, col(0), col(1), EPS2)
            C2Rv = C2R.rearrange("p nb (s y x) -> p (nb s) y x", s=2, x=16)
            for h in range(2):
                nc.scalar.activation(
                    out=H2[:, h * 16:(h + 1) * 16, 1:17, 1:17],
                    in_=C2Rv[:, h * 16:(h + 1) * 16],
                    func=AF.Prelu, bias=d2[:, 0:1], scale=a2[:, 0:1], alpha=0.2)

            # ---- conv3: PSUM-resident per mc; evict only stats + own 4 ----
            H2v = H2.rearrange("p s (r a) (v c) -> p s r a v c", a=2, c=2)
            s3ch = sm.tile([128, 2, 4, 6], f32, tag="s3ch")
            for mc in range(2):
                tiles = []
                for half in range(2):
                    pt3 = big.tile([128, 1024], f32, tag="mm")
                    tiles.append(pt3)
                    k = 0
                    for dy in range(4):
                        for dx in range(4):
                            for q in range(2):
                                s0 = half * 16 + q * 8
                                nc.tensor.matmul(
                                    pt3[:, q * 512:(q + 1) * 512],
                                    pk_at("c3w",
                                          ((dy * 4 + dx) * 2 + mc) * 128, 128),
                                    H2v[:, s0:s0 + 8, dy // 2:dy // 2 + 8,
                                        dy % 2, dx // 2:dx // 2 + 8, dx % 2],
                                    start=(k == 0), stop=(k == 15),
                                )
                            k += 1
                    for q in range(2):
                        nc.vector.bn_stats(
                            out=s3ch[:, mc, 2 * half + q, :],
                            in_=pt3[:, q * 512:(q + 1) * 512])
                mv3 = sm.tile([128, 2], f32, tag="mv3")
                nc.vector.bn_aggr(out=mv3,
                                  in_=s3ch[:, mc].rearrange("p c s -> p (c s)"))
                a3, d3 = bn_finalize_mv(mv3, col(2 + mc), col(4 + mc), EPS3)
                nc.scalar.activation(
                    out=ENC[:, mc, :, :],
                    in_=tiles[0][:, 0:BPC * 64].rearrange(
                        "p (s x) -> p s x", s=BPC),
                    func=AF.Prelu, bias=d3[:, 0:1], scale=a3[:, 0:1], alpha=0.2)

        def rn_part(col, relu_bias, AUXT):
            for s in range(BPC):
                nc.vector.tensor_copy(excl[0:AUX, :],
                                      AUXT[:, s:s + 1].to_broadcast((AUX, 64)))
                acp = hold.tile([128, 512], f32, tag="hold")
                nc.tensor.matmul(acp[0:64, 0:256], ENC[:, 0, s, :],
                                 pk_at("w1t", 0, 256), start=True, stop=False)
                nc.tensor.matmul(acp[0:64, 0:256], ENC[:, 1, s, :],
                                 pk_at("w1t", 256, 256), start=False, stop=False)
                nc.tensor.matmul(acp[0:64, 0:256], pk_at("aro", 0, 64, rows=2),
                                 pk_at("exA", 0, 256, rows=2),
                                 start=False, stop=True)
                nc.tensor.matmul(acp[64:128, 0:256], ENC[:, 0, s, :],
                                 pk_at("w1b", 0, 256), start=True, stop=False)
                nc.tensor.matmul(acp[64:128, 0:256], ENC[:, 1, s, :],
                                 pk_at("w1b", 256, 256), start=False, stop=False)
                nc.tensor.matmul(acp[64:128, 0:256], excl,
                                 pk_at("exC", 0, 256, rows=11),
                                 start=False, stop=True)
                if rn_fp8:
                    ac8 = sm.tile([128, 2, 256], f8, tag="ac8")
                    nc.scalar.copy(ac8[:, 0, :], acp[:, 0:256])
                    nc.vector.scalar_tensor_tensor(
                        out=ac8[:, 1, :], in0=acp[:, 0:256], scalar=1.0,
                        in1=ac8[:, 0, :], op0=OP.mult, op1=OP.subtract)
                else:
                    ac = sm.tile([128, 256], f16, tag="ac")
                    nc.scalar.copy(ac, acp[:, 0:256])

                b0, b1, b2 = bufs
                # L1: S.T = AC.T @ G, relu -> b0 (mc outer: stationary reuse)
                for mc in range(2):
                    for t2 in range(4):
                        pt = big.tile([128, 1024], f32, tag="mm")
                        for h in range(2):
                            tt = t2 * 2 + h
                            if rn_fp8:
                                nc.tensor.matmul(
                                    pt[:, h * 512:(h + 1) * 512],
                                    ac8[:, :, mc * 128:(mc + 1) * 128],
                                    gsel8_at(tt),
                                    start=True, stop=True, perf_mode=DR)
                            else:
                                nc.tensor.matmul(
                                    pt[:, h * 512:(h + 1) * 512],
                                    ac[:, mc * 128:(mc + 1) * 128],
                                    pk_at("gsel", tt * 512, 512),
                                    start=True, stop=True)
                        relu_bias((0, 1, 0, 1, 0, 1, 0, 1)[mc * 4 + t2],
                                  b0[:, mc, t2 * 1024:(t2 + 1) * 1024], pt,
                                  None)
                # L2..L4 dual-fp8 DoubleRow (or kc-outer fp16)
                for li, (wnm, bci, src, dst) in enumerate((
                        ("gw2", 6, b0, b1), ("gw3", 8, b1, b2),
                        ("gw4", 10, b2, None))):
                    if li == 2:
                        p4 = sm.tile([128, 2, 4], f32, tag="p4")
                    for mc in range(2):
                        for t2 in range(4):
                            pt = big.tile([128, 1024], f32, tag="mm")
                            for h in range(2):
                                tt = t2 * 2 + h
                                for kc in range(2):
                                    if rn_fp8:
                                        nc.tensor.matmul(
                                            pt[:, h * 512:(h + 1) * 512],
                                            pk8_dual(wnm, kc, mc),
                                            bcast2(src[:, kc,
                                                       tt * 512:(tt + 1) * 512]),
                                            start=(kc == 0), stop=(kc == 1),
                                            perf_mode=DR)
                                    else:
                                        nc.tensor.matmul(
                                            pt[:, h * 512:(h + 1) * 512],
                                            pk_at(wnm,
                                                  (kc * 2 + mc) * 128, 128),
                                            src[:, kc,
                                                tt * 512:(tt + 1) * 512],
                                            start=(kc == 0), stop=(kc == 1))
                            sc = INV_SG if rn_fp8 else None
                            if li < 2:
                                relu_bias((0, 1, 0, 1, 0, 0, 1, 0)[mc * 4 + t2],
                                          dst[:, mc,
                                              t2 * 1024:(t2 + 1) * 1024],
                                          pt, col(bci + mc), scale=sc)
                            else:
                                tb = trash.tile([128, 1024], f16, tag="tr4")
                                relu_bias((0, 1, 0, 1, 0, 1, 0, 1)[mc * 4 + t2],
                                          tb, pt, col(bci + mc),
                                          accum=p4[:, mc, t2:t2 + 1], scale=sc)
                nc.vector.tensor_reduce(out=POOL[:, s, :], in_=p4,
                                        axis=mybir.AxisListType.X, op=OP.add)

            # ---- f MLP ----
            POOLh = st.tile([128, BPC, 2], f16, tag="poolh")
            nc.vector.tensor_copy(POOLh, POOL)
            F1 = st.tile([128, 2, BPC], f16, tag="f1")
            F2 = st.tile([128, 2, BPC], f16, tag="f2")
            for (wnm, bci, src, dst) in (("fw1", 12, POOLh, F1),
                                         ("fw2", 14, F1, F2)):
                for mc in range(2):
                    pt = hold.tile([128, 512], f32, tag="hold")
                    if src is POOLh:
                        r0, r1 = src[:, :, 0], src[:, :, 1]
                    else:
                        r0, r1 = src[:, 0, :], src[:, 1, :]
                    nc.tensor.matmul(pt[:, 0:BPC], pk_at(wnm, mc * 128, 128),
                                     r0, start=True, stop=False)
                    nc.tensor.matmul(pt[:, 0:BPC],
                                     pk_at(wnm, (2 + mc) * 128, 128),
                                     r1, start=False, stop=True)
                    relu_bias(0, dst[:, mc, :], pt[:, 0:BPC], col(bci + mc))
            pt = hold.tile([128, 512], f32, tag="hold")
            nc.tensor.matmul(pt[0:1, 0:BPC], pk_at("fw3", 0, 1), F2[:, 0, :],
                             start=True, stop=False)
            nc.tensor.matmul(pt[0:1, 0:BPC], pk_at("fw3", 1, 1), F2[:, 1, :],
                             start=False, stop=True)
            osb = sm.tile([1, BPC], f32, tag="osb")
            nc.scalar.activation(out=osb, in_=pt[0:1, 0:BPC], func=AF.Identity,
                                 bias=fb3_s[0:1, 0:1])
            nc.sync.dma_start(out=out[:, :], in_=osb)

        def body():
            nc.sync.dma_start(out=PK, in_=pk16.ap())
            nc.sync.dma_start(out=P32, in_=pk32.ap())
            nc.gpsimd.dma_start(out=PK8, in_=pk8d.ap())
            nc.sync.dma_start(out=excl[10:11, :],
                              in_=pk_at("aro", 0, 64, rows=1))
            for H1 in H1s:
                nc.vector.memset(H1[:, :, 0:1, :], 0.0)
                nc.vector.memset(H1[:, :, 33:34, :], 0.0)
                nc.vector.memset(H1[:, :, :, 0:1], 0.0)
                nc.vector.memset(H1[:, :, :, 33:34], 0.0)
                nc.vector.memset(H1[64:128, :, :, 32:33], 0.0)
            nc.vector.memset(H2[:, :, 0:1, :], 0.0)
            nc.vector.memset(H2[:, :, 17:18, :], 0.0)
            nc.vector.memset(H2[:, :, :, 0:1], 0.0)
            nc.vector.memset(H2[:, :, :, 17:18], 0.0)

            # consts transpose -> CONST_T [128, 20]
            CONST_T = st.tile([128, 20], f32, tag="constT")
            ptc = hold.tile([128, 512], f32, tag="hold")
            nc.tensor.transpose(ptc[:, 0:20], csb, id_s[0:20, 0:20])
            nc.vector.tensor_copy(CONST_T, ptc[:, 0:20])
            AUXT = st.tile([AUX, BPC], f16, tag="auxT")
            pta = hold.tile([128, 512], f32, tag="hold")
            nc.tensor.transpose(pta[0:AUX, 0:BPC], asb, id_s[0:BPC, 0:BPC])
            nc.vector.tensor_copy(AUXT, pta[0:AUX, 0:BPC])

            def col(i):
                return CONST_T[:, i:i + 1]

            zcol = sm.tile([128, 1], f32, tag="zcol")
            nc.vector.memset(zcol, 0.0)

            def relu_bias(eng, dst, psum, bias_col, accum=None, scale=None):
                # eng: 0=ACT, 1=DVE. relu(psum*scale + bias); the DVE path
                # with scale drops bias (asserted zero host-side).
                if eng is True:
                    eng = 0
                elif eng is False:
                    eng = 1
                if eng == 0:
                    nc.scalar.activation(
                        out=dst, in_=psum, func=AF.Relu,
                        bias=bias_col if bias_col is not None else 0.0,
                        scale=scale if scale is not None else 1.0,
                        accum_out=accum)
                elif accum is not None:
                    if scale is not None:
                        nc.vector.scalar_tensor_tensor(
                            out=dst, in0=psum, scalar=scale,
                            in1=zcol[:, 0:1].to_broadcast(
                                (psum.partition_size(), psum.free_size())),
                            op0=OP.mult, op1=OP.max, accum_out=accum)
                    else:
                        nc.vector.scalar_tensor_tensor(
                            out=dst, in0=psum,
                            scalar=bias_col if bias_col is not None else 0.0,
                            in1=zcol[:, 0:1].to_broadcast(
                                (psum.partition_size(), psum.free_size())),
                            op0=OP.add, op1=OP.max, accum_out=accum)
                else:
                    if scale is not None:
                        nc.vector.tensor_scalar(
                            out=dst, in0=psum, scalar1=scale,
                            scalar2=0.0, op0=OP.mult, op1=OP.max)
                    else:
                        nc.vector.tensor_scalar(
                            out=dst, in0=psum,
                            scalar1=bias_col if bias_col is not None else 0.0,
                            scalar2=0.0, op0=OP.add, op1=OP.max)

            if run_conv:
                conv_part(col)
            else:
                nc.vector.memset(ENC, 0.02)
            if run_rn:
                rn_part(col, relu_bias, AUXT)
            else:
                osb = sm.tile([1, BPC], f32, tag="osb")
                nc.vector.tensor_copy(osb, ENC[0:1, 0, :, 0])
                nc.sync.dma_start(out=out[:, :], in_=osb)

        if loop_n > 0:
            with tc.For_i(0, loop_n, 1):
                body()
        else:
            body()

    nc.compile()
    return nc


def _host_im2col(img):
    """img [B, 12, 64, 64] -> imc [96, B, 32, 66] fp8 (scaled by S_IMG).
    imc[dy*12+ic, s, oy, 1+u] = img[s, ic, 2*oy+dy-1, u] (j=0 rows),
    rows 48:96 = rows 0:48 shifted one column left (j=1), zero-padded."""
    img8 = np.asarray(img, np.float16)
    imc = np.zeros((96, B, 32, 66), np.float16)
    oy = np.arange(32)
    for dy in range(4):
        h = 2 * oy + dy - 1
        ok = (h >= 0) & (h < 64)
        # [s, ic, oy_ok, u] -> [ic, s, oy_ok, u]
        imc[dy * 12:(dy + 1) * 12, :, ok, 1:65] = \
            img8[:, :, h[ok], :].transpose(1, 0, 2, 3)
    imc[48:96, :, :, 0:65] = imc[0:48, :, :, 1:66]
    return imc


def _in_maps(inputs):
    pk, pk8, pk32 = _host_pack(inputs)
    img = np.asarray(inputs['input_image'], dtype=np.float32)
    aux = np.asarray(inputs['inputg_aux_v'], dtype=np.float32)
    imc0 = _host_im2col(img)
    in_maps = []
    for c in range(NCORES):
        own = list(range(c * BPC, (c + 1) * BPC))
        rest = [i for i in range(B) if i not in own]
        perm = own + rest
        p32c = pk32.copy()
        p32c[0:BPC, 161:161 + AUX] = aux[own]
        in_maps.append({'imc': np.ascontiguousarray(imc0[:, perm]),
                        'pk16': pk, 'pk8': pk8, 'pk32': p32c})
    return in_maps


def kernel(**inputs):
    from concourse.bass_utils import run_bass_kernel_spmd

    key = ("nc3", NCORES)
    if key not in _CACHE:
        _CACHE[key] = _build(NCORES)
    nc = _CACHE[key]

    in_maps = _in_maps(inputs)
    res = run_bass_kernel_spmd(nc, in_maps, core_ids=list(range(NCORES)))
    kernel.last_result = res
    outs = [res.results[c]["out"].reshape(BPC, 1) for c in range(NCORES)]
    return np.concatenate(outs, axis=0).astype(np.float32)
